# revision 34
# baseline (speedup 1.0000x reference)
"""Trainium2 Bass kernel for nn_Attention_17008070493108.

Dense transformer attention block: QKV proj -> per-head LayerNorm -> RoPE
-> SDPA -> out proj, for x[2, 2048, 1024], H=16 heads, head_dim=64.

Sharding: tensor-parallel over heads. Each of the 8 NeuronCores owns 2
heads end-to-end (QKV column slices, norm, RoPE, attention). The
per-head context vectors are exchanged with a single AllToAll so each
core finishes the output projection (contraction over the full 1024
model dims) for its own 512-row slice of the output; the host
concatenates row slices.

Layouts (per core):
  xT        [1024, 4096]  model-dim on partitions (host-transposed x)
  QT/KT/VT  [128, 4096]   2 heads stacked on partitions (hA 0:63, hB 64:127)
  scoresT   [128 keys, q] key tiles on partitions; softmax denominator via
                          a ones-column appended to V (ctx_aug row 64)
  ctxT      [128, 4096]   -> AllToAll -> out rows [512, 1024]

Matmul dtype is float32r (fp32 rounded to 11-bit mantissa, full PE rate).
Per-row scalars (LN stats, softmax denominators) are staged through DRAM
bounces because engine SBUF accesses need 32-aligned partition bases.
"""

import numpy as np

from concourse import bacc, tile, mybir
from concourse.bass_utils import run_bass_kernel_spmd

# ---------------------------------------------------------------- constants
DIM = 1024
H = 16
HD = 64
B = 2
N = 2048
R = B * N          # 4096 flattened rows
NCORE = 8
EPS = 1e-6

F32 = mybir.dt.float32
F32R = mybir.dt.float32r
ADD = mybir.AluOpType.add
SUB = mybir.AluOpType.subtract
MUL = mybir.AluOpType.mult

RC = R // 512        # 8 row chunks of 512
KT_DIM = DIM // 128  # 8 contraction tiles for the projections
NQC = N // 512       # 4 q chunks per batch
NKT = N // 128       # 16 key tiles per batch
VSTRIDE = 130        # per-keytile V_aug block: [vA(64) | 1 | vB(64) | 1]

# When True, build() adds intermediate tensors as extra outputs (used by
# test.py for stage-level verification; off for normal runs).
DEBUG_OUTPUTS = False


def _round_fp32r(x: np.ndarray) -> np.ndarray:
    """Round fp32 to fp32r (11-bit mantissa, RNE) so DMA-loaded matmul
    operands satisfy the FP32r rounding requirement."""
    u = np.ascontiguousarray(x, dtype=np.float32).view(np.uint32)
    lsb = (u >> np.uint32(12)) & np.uint32(1)
    r = (u + np.uint32(0x7FF) + lsb) & np.uint32(0xFFFFF000)
    return r.view(np.float32)


# ---------------------------------------------------------------- graph
def build():
    nc = bacc.Bacc("TRN2", target_bir_lowering=False, debug=False,
                   num_devices=NCORE)

    # ---- DRAM parameters
    xT_d = nc.dram_tensor("xT", [DIM, R], F32R, kind="ExternalInput")
    wqkv_d = nc.dram_tensor("wqkv", [DIM, 3 * 128], F32R, kind="ExternalInput")
    bqkv_d = nc.dram_tensor("bqkv", [3, 128, 1], F32, kind="ExternalInput")
    onesblk_d = nc.dram_tensor("onesblk", [RC, 128, 16], F32R,
                               kind="ExternalInput")
    wbln_d = nc.dram_tensor("wbln", [4, 128, 1], F32, kind="ExternalInput")
    cos_d = nc.dram_tensor("cosr", [128, R], F32, kind="ExternalInput")
    sinm_d = nc.dram_tensor("sinm", [128, R], F32, kind="ExternalInput")
    ident_d = nc.dram_tensor("ident", [128, 128], F32, kind="ExternalInput")
    ones_d = nc.dram_tensor("ones64", [128, 4 * NKT], F32R,
                            kind="ExternalInput")
    wo_d = nc.dram_tensor("wo", [DIM, DIM], F32R, kind="ExternalInput")
    borep_d = nc.dram_tensor("borep", [128, DIM], F32, kind="ExternalInput")
    out_d = nc.dram_tensor("out", [R // NCORE, DIM], F32, kind="ExternalOutput")
    if DEBUG_OUTPUTS:
        dbg_qrot = nc.dram_tensor("dbg_qrot", [128, R], F32,
                                  kind="ExternalOutput")
        dbg_krot = nc.dram_tensor("dbg_krot", [128, R], F32,
                                  kind="ExternalOutput")
        dbg_den = nc.dram_tensor("dbg_den", [16, 512], F32,
                                 kind="ExternalOutput")
        dbg_ctxn = nc.dram_tensor("dbg_ctxn", [128, R], F32,
                                  kind="ExternalOutput")
        dbg_qproj = nc.dram_tensor("dbg_qproj", [128, R], F32,
                                   kind="ExternalOutput")
        dbg_rstd = nc.dram_tensor("dbg_rstd", [16, 512], F32,
                                  kind="ExternalOutput")
        dbg_mur = nc.dram_tensor("dbg_mur", [16, 512], F32,
                                 kind="ExternalOutput")
        dbg_rep = nc.dram_tensor("dbg_rep", [128, 512], F32,
                                 kind="ExternalOutput")
        dbg_tn = nc.dram_tensor("dbg_tn", [128, 512], F32,
                                kind="ExternalOutput")
        dbg_vaug = nc.dram_tensor("dbg_vaug", [128, 2 * NKT * VSTRIDE], F32,
                                  kind="ExternalOutput")

    with tile.TileContext(nc) as tc:
        with (
            tc.tile_pool(name="const", bufs=1) as cpool,
            tc.tile_pool(name="persist", bufs=1) as ppool,
            tc.tile_pool(name="dram", bufs=1, space="DRAM") as dpool,
        ):
            # ---- constants in SBUF
            wqkv_sb = cpool.tile([128, KT_DIM, 384], F32R)
            nc.sync.dma_start(
                out=wqkv_sb[:],
                in_=wqkv_d.ap().rearrange("(k p) c -> p k c", p=128))
            bq_sb = cpool.tile([128, 1], F32)
            bk_sb = cpool.tile([128, 1], F32)
            bv_sb = cpool.tile([128, 1], F32)
            nc.sync.dma_start(out=bq_sb[:], in_=bqkv_d.ap()[0])
            nc.sync.dma_start(out=bk_sb[:], in_=bqkv_d.ap()[1])
            nc.sync.dma_start(out=bv_sb[:], in_=bqkv_d.ap()[2])
            onesblk_sb = cpool.tile([128, RC, 16], F32R)
            nc.sync.dma_start(
                out=onesblk_sb[:],
                in_=onesblk_d.ap().rearrange("j p c -> p j c"))
            wlnq_sb = cpool.tile([128, 1], F32)
            blnq_sb = cpool.tile([128, 1], F32)
            wlnk_sb = cpool.tile([128, 1], F32)
            blnk_sb = cpool.tile([128, 1], F32)
            nc.sync.dma_start(out=wlnq_sb[:], in_=wbln_d.ap()[0])
            nc.sync.dma_start(out=blnq_sb[:], in_=wbln_d.ap()[1])
            nc.sync.dma_start(out=wlnk_sb[:], in_=wbln_d.ap()[2])
            nc.sync.dma_start(out=blnk_sb[:], in_=wbln_d.ap()[3])
            ident_sb = cpool.tile([128, 128], F32)
            nc.sync.dma_start(out=ident_sb[:], in_=ident_d.ap()[:, :])
            borep_sb = cpool.tile([128, DIM], F32)
            nc.sync.dma_start(out=borep_sb[:], in_=borep_d.ap()[:, :])

            # ---- persistent tensors; qraw/kraw are overwritten in place
            # by the normalized+roped values during phase 2.
            qraw = ppool.tile([128, R], F32R, tag="qraw")
            kraw = ppool.tile([128, R], F32R, tag="kraw")
            vaug = ppool.tile([128, 2 * NKT * VSTRIDE], F32R, tag="vaug")
            # normalized context per head, both at partition base 0 (DVE
            # lanes cannot cross partitions; ctx_aug tiles sit at base 0)
            ctxn_a = ppool.tile([64, R], F32R, tag="ctxn_a")
            ctxn_b = ppool.tile([64, R], F32R, tag="ctxn_b")

            # ones columns of V_aug (col 64 of each 65-wide block), via DMA
            nc.sync.dma_start(
                out=vaug[:].rearrange("p (k c) -> p k c", c=65)[:, :, 64:65],
                in_=ones_d.ap()[:, :])

            # ================= Phase 1: QKV projection =================
            # QT/KT[p, r] = sum_k Wqkv[k, m*128+p] * x[r, k] + b
            # V goes through a PE transpose into keys-on-partitions V_aug.
            # Interleaved emission: V proj, then per-tensor (proj ->
            # stats -> LN+RoPE apply) so DVE-heavy LN overlaps the next
            # projection on PE. Each projection re-streams xT.
            with (
                tc.tile_pool(name="xtp", bufs=10) as xtpool,
                tc.tile_pool(name="vchp", bufs=3) as vchpool,
                tc.tile_pool(name="chp", bufs=2) as chpool,
                tc.tile_pool(name="statp", bufs=8) as statpool,
                tc.tile_pool(name="stagp", bufs=4) as stagpool,
                tc.tile_pool(name="ps1", bufs=2, space="PSUM") as ps1,
                tc.tile_pool(name="ps1v", bufs=2, space="PSUM") as ps1v,
                tc.tile_pool(name="ps2", bufs=2, space="PSUM") as ps2,
            ):
                def emit_proj(m, dest, bias):
                    for r in range(RC):
                        xts = []
                        for kt in range(KT_DIM):
                            xt = xtpool.tile([128, 512], F32R, tag="xt",
                                             name=f"xt_{m}_{r}_{kt}")
                            nc.sync.dma_start(
                                out=xt[:],
                                in_=xT_d.ap()[kt * 128:(kt + 1) * 128,
                                              r * 512:(r + 1) * 512])
                            xts.append(xt)
                        ps = ps1.tile([128, 512], F32, tag="proj",
                                      name=f"proj_{m}_{r}")
                        for kt in range(KT_DIM):
                            nc.tensor.matmul(
                                ps[:],
                                wqkv_sb[:, kt, m * 128:(m + 1) * 128],
                                xts[kt][:],
                                start=(kt == 0), stop=(kt == KT_DIM - 1))
                        if dest is not None:
                            nc.vector.tensor_scalar(
                                dest[:, r * 512:(r + 1) * 512], ps[:],
                                bias[:], None, ADD)
                        else:
                            vch = vchpool.tile([128, 512], F32, tag="vch",
                                               name=f"vch_{r}")
                            nc.vector.tensor_scalar(vch[:], ps[:], bias[:],
                                                    None, ADD)
                            for sseg in range(4):
                                kt_glob = r * 4 + sseg
                                tps = ps1v.tile([128, 128], F32, tag="vtr",
                                                name=f"vtr_{kt_glob}")
                                nc.tensor.transpose(
                                    tps[:],
                                    vch[:, sseg * 128:(sseg + 1) * 128],
                                    ident_sb[:])
                                vb = kt_glob * VSTRIDE
                                nc.vector.tensor_copy(
                                    vaug[:, vb:vb + 64], tps[:, 0:64])
                                nc.vector.tensor_copy(
                                    vaug[:, vb + 65:vb + 129],
                                    tps[:, 64:128])

                def emit_stats(name, traw):
                    xs_ps = ps2.tile([16, 512], F32, tag="stat",
                                     name=f"xs_{name}")
                    qs_ps = ps2.tile([16, 512], F32, tag="stat",
                                     name=f"qs_{name}")
                    for j in range(RC):
                        nc.tensor.matmul(
                            xs_ps[:], onesblk_sb[:, j, :],
                            traw[:, j * 512:(j + 1) * 512],
                            start=(j == 0), stop=(j == RC - 1))
                    for j in range(RC):
                        sqc = chpool.tile([128, 512], F32R, tag="sqc",
                                          name=f"sqc_{name}_{j}")
                        nc.vector.tensor_tensor(
                            sqc[:],
                            traw[:, j * 512:(j + 1) * 512].bitcast(F32),
                            traw[:, j * 512:(j + 1) * 512].bitcast(F32),
                            MUL)
                        nc.tensor.matmul(
                            qs_ps[:], onesblk_sb[:, j, :], sqc[:],
                            start=(j == 0), stop=(j == RC - 1))
                    mu = statpool.tile([16, 512], F32, tag="stat_sb",
                                       name=f"mu_{name}")
                    msqe = statpool.tile([16, 512], F32, tag="stat_sb",
                                         name=f"msqe_{name}")
                    nc.vector.tensor_scalar(mu[:], xs_ps[:], 1.0 / HD,
                                            None, MUL)
                    nc.vector.tensor_scalar(msqe[:], qs_ps[:], 1.0 / HD,
                                            EPS, MUL, ADD)
                    var = statpool.tile([16, 512], F32, tag="stat_sb",
                                        name=f"var_{name}")
                    nc.vector.tensor_tensor(var[:], mu[:], mu[:], MUL)
                    nc.vector.tensor_tensor(var[:], msqe[:], var[:], SUB)
                    sd = statpool.tile([16, 512], F32, tag="stat_sb",
                                       name=f"sd_{name}")
                    nc.scalar.activation(sd[:], var[:],
                                         mybir.ActivationFunctionType.Sqrt)
                    rstd = statpool.tile([16, 512], F32, tag="stat_sb",
                                         name=f"rstd_{name}")
                    nc.vector.reciprocal(rstd[:], sd[:])
                    murstd = statpool.tile([16, 512], F32, tag="stat_sb",
                                           name=f"murstd_{name}")
                    nc.vector.tensor_tensor(murstd[:], mu[:], rstd[:], MUL)
                    rdr = dpool.tile([16, 512], F32, name=f"rstd_dr_{name}")
                    mdr = dpool.tile([16, 512], F32, name=f"mur_dr_{name}")
                    nc.sync.dma_start(out=rdr[:], in_=rstd[:])
                    nc.sync.dma_start(out=mdr[:], in_=murstd[:])
                    if DEBUG_OUTPUTS and name == "q":
                        nc.sync.dma_start(out=dbg_rstd.ap()[:, :],
                                          in_=rstd[:])
                        nc.sync.dma_start(out=dbg_mur.ap()[:, :],
                                          in_=murstd[:])
                    return rdr, mdr

                def emit_apply(name, traw, w_sb, b_sb, rdr, mdr):
                    for j in range(RC):
                        jsl = slice(j * 512, (j + 1) * 512)
                        cosc = chpool.tile([128, 512], F32, tag="cosc",
                                           name=f"cosc_{name}_{j}")
                        sinc = chpool.tile([128, 512], F32, tag="sinc",
                                           name=f"sinc_{name}_{j}")
                        nc.sync.dma_start(out=cosc[:], in_=cos_d.ap()[:, jsl])
                        nc.sync.dma_start(out=sinc[:],
                                          in_=sinm_d.ap()[:, jsl])
                        rep_r = chpool.tile([128, 512], F32, tag="rep_r",
                                            name=f"rep_r_{name}_{j}")
                        rep_m = chpool.tile([128, 512], F32, tag="rep_m",
                                            name=f"rep_m_{name}_{j}")
                        for h in range(2):
                            stg_r = stagpool.tile([1, 512], F32, tag="stg",
                                                  name=f"sr_{name}_{j}_{h}")
                            stg_m = stagpool.tile([1, 512], F32, tag="stg",
                                                  name=f"sm_{name}_{j}_{h}")
                            nc.sync.dma_start(out=stg_r[:],
                                              in_=rdr[2 * j + h])
                            nc.sync.dma_start(out=stg_m[:],
                                              in_=mdr[2 * j + h])
                            if h == 0:
                                nc.gpsimd.partition_broadcast(
                                    rep_r[0:64, :], stg_r[:], channels=64)
                                nc.gpsimd.partition_broadcast(
                                    rep_m[0:64, :], stg_m[:], channels=64)
                            else:
                                tmp_r = stagpool.tile(
                                    [64, 512], F32, tag="tmpb",
                                    name=f"tr_{name}_{j}")
                                tmp_m = stagpool.tile(
                                    [64, 512], F32, tag="tmpb",
                                    name=f"tm_{name}_{j}")
                                nc.gpsimd.partition_broadcast(
                                    tmp_r[:], stg_r[:], channels=64)
                                nc.gpsimd.partition_broadcast(
                                    tmp_m[:], stg_m[:], channels=64)
                                nc.sync.dma_start(out=rep_r[64:128, :],
                                                  in_=tmp_r[:])
                                nc.sync.dma_start(out=rep_m[64:128, :],
                                                  in_=tmp_m[:])
                        tn = chpool.tile([128, 512], F32, tag="tn",
                                         name=f"tn_{name}_{j}")
                        nc.vector.tensor_tensor(
                            tn[:], traw[:, jsl].bitcast(F32), rep_r[:], MUL)
                        nc.vector.tensor_tensor(tn[:], tn[:], rep_m[:], SUB)
                        nc.vector.tensor_scalar(tn[:], tn[:], w_sb[:],
                                                b_sb[:], MUL, ADD)
                        if DEBUG_OUTPUTS and name == "q" and j == 0:
                            nc.sync.dma_start(out=dbg_rep.ap()[:, :],
                                              in_=rep_r[:])
                            nc.sync.dma_start(out=dbg_tn.ap()[:, :],
                                              in_=tn[:])
                        swp = chpool.tile([128, 512], F32, tag="swp",
                                          name=f"swp_{name}_{j}")
                        for (dst, src) in ((0, 32), (32, 0), (64, 96),
                                           (96, 64)):
                            nc.sync.dma_start(out=swp[dst:dst + 32, :],
                                              in_=tn[src:src + 32, :])
                        t1 = chpool.tile([128, 512], F32, tag="t1",
                                         name=f"t1_{name}_{j}")
                        nc.vector.tensor_tensor(t1[:], tn[:], cosc[:], MUL)
                        nc.vector.tensor_tensor(swp[:], swp[:], sinc[:], MUL)
                        nc.vector.tensor_tensor(traw[:, jsl], t1[:],
                                                swp[:], ADD)

                emit_proj(2, None, bv_sb)
                emit_proj(0, qraw, bq_sb)
                if DEBUG_OUTPUTS:
                    nc.sync.dma_start(out=dbg_qproj.ap()[:, :],
                                      in_=qraw[:].bitcast(F32))
                    nc.sync.dma_start(out=dbg_vaug.ap()[:, :],
                                      in_=vaug[:].bitcast(F32))
                rdr_q, mdr_q = emit_stats("q", qraw)
                emit_apply("q", qraw, wlnq_sb, blnq_sb, rdr_q, mdr_q)
                emit_proj(1, kraw, bk_sb)
                rdr_k, mdr_k = emit_stats("k", kraw)
                emit_apply("k", kraw, wlnk_sb, blnk_sb, rdr_k, mdr_k)

            qrot, krot = qraw, kraw  # now hold the normalized+roped values

            # ================= Phase 3: SDPA =================
            # per batch g, per q-chunk: scoresT tiles [128 keys, 512 q] for
            # both heads side by side in a 2-bank psum tile; exp on ACT;
            # PV accumulates ctx_aug [65, 512] (row 64 = softmax denom).
            den_dr = dpool.tile([16, 512], F32, name="den_dr")
            rec_dr = dpool.tile([16, 512], F32, name="rec_dr")
            denpacks = []
            with (
                tc.tile_pool(name="expp", bufs=3) as exppool,
                tc.tile_pool(name="ctxup", bufs=16) as ctxupool,
                tc.tile_pool(name="sp3", bufs=4) as sp3,
                tc.tile_pool(name="ps_sc", bufs=2, space="PSUM") as ps_sc,
                tc.tile_pool(name="ps_ctx", bufs=4, space="PSUM") as ps_ctx,
            ):
                for g in range(B):
                    ctxu_tiles = {}
                    for qc in range(NQC):
                        qsl = slice(g * 2048 + qc * 512,
                                    g * 2048 + (qc + 1) * 512)
                        ctxps_a = ps_ctx.tile([65, 512], F32, tag="ctx")
                        ctxps_b = ps_ctx.tile([65, 512], F32, tag="ctx")
                        for kt in range(NKT):
                            ksl = slice(g * 2048 + kt * 128,
                                        g * 2048 + (kt + 1) * 128)
                            scps = ps_sc.tile([128, 1024], F32, tag="sc")
                            nc.tensor.matmul(scps[:, 0:512],
                                             krot[0:64, ksl],
                                             qrot[0:64, qsl],
                                             start=True, stop=True,
                                             tile_position=(0, 0))
                            nc.tensor.matmul(scps[:, 512:1024],
                                             krot[64:128, ksl],
                                             qrot[64:128, qsl],
                                             start=True, stop=True,
                                             tile_position=(64, 0))
                            expt = exppool.tile([128, 1024], F32R,
                                                tag="expt")
                            nc.scalar.activation(
                                expt[:], scps[:],
                                mybir.ActivationFunctionType.Exp,
                                scale=float(HD) ** -0.5)
                            vbase = (g * NKT + kt) * VSTRIDE
                            nc.tensor.matmul(
                                ctxps_a[:], vaug[:, vbase:vbase + 65],
                                expt[:, 0:512],
                                start=(kt == 0), stop=(kt == NKT - 1))
                            nc.tensor.matmul(
                                ctxps_b[:],
                                vaug[:, vbase + 65:vbase + 130],
                                expt[:, 512:1024],
                                start=(kt == 0), stop=(kt == NKT - 1))
                        # evacuate (unnormalized) + stage denominators
                        cua = ctxupool.tile([65, 512], F32, tag="ctxu")
                        cub = ctxupool.tile([65, 512], F32, tag="ctxu")
                        nc.vector.tensor_copy(cua[:], ctxps_a[:])
                        nc.vector.tensor_copy(cub[:], ctxps_b[:])
                        idx = g * 8 + qc * 2
                        nc.sync.dma_start(out=den_dr[idx], in_=cua[64:65, :])
                        nc.sync.dma_start(out=den_dr[idx + 1],
                                          in_=cub[64:65, :])
                        ctxu_tiles[qc] = (cua, cub)

                    # ---- normalize batch g: ctxn = ctxu * (1/denom)
                    denpack = sp3.tile([8, 512], F32, tag="denpack")
                    nc.sync.dma_start(
                        out=denpack[:],
                        in_=den_dr.opt()[g * 8:(g + 1) * 8, :])
                    recip = sp3.tile([8, 512], F32, tag="recip")
                    nc.vector.reciprocal(recip[:], denpack[:])
                    nc.sync.dma_start(out=rec_dr[g * 8:(g + 1) * 8, :],
                                      in_=recip[:])
                    denpacks.append(denpack)
                    for qc in range(NQC):
                        cua, cub = ctxu_tiles[qc]
                        qsl = slice(g * 2048 + qc * 512,
                                    g * 2048 + (qc + 1) * 512)
                        for h, cu, dst in ((0, cua, ctxn_a),
                                           (1, cub, ctxn_b)):
                            stg = sp3.tile([1, 512], F32, tag="stg3")
                            nc.sync.dma_start(
                                out=stg[:], in_=rec_dr[g * 8 + qc * 2 + h])
                            rep = sp3.tile([64, 512], F32, tag="rep")
                            nc.gpsimd.partition_broadcast(rep[:], stg[:],
                                                          channels=64)
                            nc.gpsimd.tensor_tensor(
                                dst[:, qsl], cu[0:64, :], rep[:], MUL)

            # ================= Phase 4: AllToAll =================
            a2a_in = dpool.tile([NCORE, 128, 512], F32R)
            a2a_out = dpool.tile([NCORE, 128, 512], F32R)
            for j in range(NCORE):
                nc.sync.dma_start(out=a2a_in[j][0:64, :],
                                  in_=ctxn_a[:, j * 512:(j + 1) * 512])
                nc.sync.dma_start(out=a2a_in[j][64:128, :],
                                  in_=ctxn_b[:, j * 512:(j + 1) * 512])
            nc.gpsimd.collective_compute(
                "AllToAll", mybir.AluOpType.bypass,
                ins=[a2a_in.opt()], outs=[a2a_out.opt()],
                replica_groups=[list(range(NCORE))],
            )

            # ================= Phase 5: output projection ==============
            # out[512 rows, 1024] = sum_kt ctx_all[kt][:, rows].T @ Wo[kt]
            with (
                tc.tile_pool(name="wop", bufs=3) as wopool,
                tc.tile_pool(name="sp5", bufs=4) as sp5,
                tc.tile_pool(name="ps_out", bufs=4, space="PSUM") as ps_out,
            ):
                ops = [ps_out.tile([128, 1024], F32, tag="outp",
                                   name=f"outp{i}") for i in range(4)]
                for kt in range(KT_DIM):
                    wo_sb = wopool.tile([128, DIM], F32R, tag="wo")
                    nc.sync.dma_start(
                        out=wo_sb[:],
                        in_=wo_d.ap()[kt * 128:(kt + 1) * 128, :])
                    cg = wopool.tile([128, 512], F32R, tag="ctxg")
                    nc.sync.dma_start(out=cg[:], in_=a2a_out[kt])
                    for rt in range(4):
                        for nh in range(2):
                            nc.tensor.matmul(
                                ops[rt][:, nh * 512:(nh + 1) * 512],
                                cg[:, rt * 128:(rt + 1) * 128],
                                wo_sb[:, nh * 512:(nh + 1) * 512],
                                start=(kt == 0), stop=(kt == KT_DIM - 1))
                for rt in range(4):
                    osb = sp5.tile([128, DIM], F32, tag="osb")
                    nc.vector.tensor_tensor(osb[:], ops[rt][:], borep_sb[:],
                                            ADD)
                    nc.sync.dma_start(
                        out=out_d.ap()[rt * 128:(rt + 1) * 128, :],
                        in_=osb[:])

            if DEBUG_OUTPUTS:
                nc.sync.dma_start(out=dbg_qrot.ap()[:, :],
                                  in_=qrot[:].bitcast(F32))
                nc.sync.dma_start(out=dbg_krot.ap()[:, :],
                                  in_=krot[:].bitcast(F32))
                nc.sync.dma_start(out=dbg_den.ap()[:, :], in_=den_dr.opt())
                nc.sync.dma_start(out=dbg_ctxn.ap()[0:64, :],
                                  in_=ctxn_a[:].bitcast(F32))
                nc.sync.dma_start(out=dbg_ctxn.ap()[64:128, :],
                                  in_=ctxn_b[:].bitcast(F32))

    nc.compile()
    return nc


# ---------------------------------------------------------------- host side
def prepare_in_maps(x, rotary_cos, rotary_sin, Wq, bq, Wk, bk, Wv, bv,
                    q_norm_w, q_norm_b, k_norm_w, k_norm_b, Wo, bo):
    x = np.asarray(x, np.float32)
    xT = _round_fp32r(np.ascontiguousarray(x.reshape(R, DIM).T))

    Wcat = np.concatenate([np.asarray(Wq, np.float32),
                           np.asarray(Wk, np.float32),
                           np.asarray(Wv, np.float32)], axis=1)
    bcat = np.concatenate([np.asarray(bq, np.float32),
                           np.asarray(bk, np.float32),
                           np.asarray(bv, np.float32)])

    # faithful packed-qkv permutation: head h uses concat cols
    # [192h, 192h+64) as q, +64.. as k, +128.. as v
    def head_cols(h, part):
        s = 192 * h + 64 * part
        return np.arange(s, s + 64)

    cos_flat = np.asarray(rotary_cos, np.float32).reshape(R, HD).T  # [64, R]
    sin_flat = np.asarray(rotary_sin, np.float32).reshape(R, HD).T
    sinm = sin_flat.copy()
    sinm[0:32] = -sin_flat[0:32]
    cos_rep = np.ascontiguousarray(np.tile(cos_flat, (2, 1)))       # [128, R]
    sinm_rep = np.ascontiguousarray(np.tile(sinm, (2, 1)))

    onesblk = np.zeros((RC, 128, 16), np.float32)
    for j in range(RC):
        onesblk[j, 0:64, 2 * j] = 1.0
        onesblk[j, 64:128, 2 * j + 1] = 1.0

    wbln = np.stack([
        np.tile(np.asarray(q_norm_w, np.float32), 2)[:, None],
        np.tile(np.asarray(q_norm_b, np.float32), 2)[:, None],
        np.tile(np.asarray(k_norm_w, np.float32), 2)[:, None],
        np.tile(np.asarray(k_norm_b, np.float32), 2)[:, None],
    ])

    ident = np.eye(128, dtype=np.float32)
    ones64 = np.ones((128, 4 * NKT), np.float32)
    borep = np.tile(np.asarray(bo, np.float32)[None, :], (128, 1))
    wo_r = _round_fp32r(np.asarray(Wo, np.float32))

    in_maps = []
    for c in range(NCORE):
        hA, hB = 2 * c, 2 * c + 1
        cols = np.concatenate([
            head_cols(hA, 0), head_cols(hB, 0),   # qA qB
            head_cols(hA, 1), head_cols(hB, 1),   # kA kB
            head_cols(hA, 2), head_cols(hB, 2),   # vA vB
        ])
        wqkv_c = _round_fp32r(np.ascontiguousarray(Wcat[:, cols]))
        bqkv_c = np.ascontiguousarray(bcat[cols].reshape(3, 128, 1))
        in_maps.append({
            "xT": xT,
            "wqkv": wqkv_c,
            "bqkv": bqkv_c,
            "onesblk": onesblk,
            "wbln": wbln,
            "cosr": cos_rep,
            "sinm": sinm_rep,
            "ident": ident,
            "ones64": ones64,
            "wo": wo_r,
            "borep": borep,
        })
    return in_maps


def assemble_output(results):
    out = np.empty((R, DIM), np.float32)
    for c in range(NCORE):
        out[c * 512:(c + 1) * 512] = results[c]["out"]
    return out.reshape(B, N, DIM)


_NC_CACHE = []


def kernel(**inputs) -> np.ndarray:
    if not _NC_CACHE:
        _NC_CACHE.append(build())
    nc = _NC_CACHE[0]
    in_maps = prepare_in_maps(**inputs)
    res = run_bass_kernel_spmd(nc, in_maps, core_ids=list(range(NCORE)))
    return assemble_output(res.results)


# revision 39
# speedup vs baseline: 1.0850x; 1.0850x over previous
"""Trainium2 Bass kernel for nn_Attention_17008070493108.

Dense transformer attention block: QKV proj -> per-head LayerNorm -> RoPE
-> SDPA -> out proj, for x[2, 2048, 1024], H=16 heads, head_dim=64.

Sharding: tensor-parallel over heads. Each of the 8 NeuronCores owns 2
heads end-to-end (QKV column slices, norm, RoPE, attention). The
per-head context vectors are exchanged with a single AllToAll so each
core finishes the output projection (contraction over the full 1024
model dims) for its own 512-row slice of the output; the host
concatenates row slices.

Layouts (per core):
  xT          [1024, 4096] model-dim on partitions (host-transposed x)
  QT/KT       [128, 2048]x2 (batch-split) heads stacked on partitions
  scoresT     [128 keys, q] key tiles on partitions; softmax denominator
                          via a ones-column appended to V (ctx_aug row 64)
  ctx         [64, 4096]x2 -> AllToAll (bf16) -> out rows [512, 1024]

The emission is software-pipelined: LayerNorm+RoPE of batch 0 overlaps
the batch-1 projections on PE, and SDPA of batch 0 overlaps the batch-1
LayerNorm on DVE. Matmuls run in float32r (fp32 with 11-bit mantissa,
full PE rate); the output projection runs in bf16.
"""

import numpy as np

from concourse import bacc, tile, mybir
from concourse.bass_utils import run_bass_kernel_spmd

# ---------------------------------------------------------------- constants
DIM = 1024
H = 16
HD = 64
B = 2
N = 2048
R = B * N          # 4096 flattened rows
NCORE = 8
EPS = 1e-6

F32 = mybir.dt.float32
F32R = mybir.dt.float32r
BF16 = mybir.dt.bfloat16
ADD = mybir.AluOpType.add
SUB = mybir.AluOpType.subtract
MUL = mybir.AluOpType.mult

RC = R // 512        # 8 row chunks of 512
KT_DIM = DIM // 128  # 8 contraction tiles for the projections
NQC = N // 512       # 4 q chunks per batch
NKT = N // 128       # 16 key tiles per batch
VSTRIDE = 130        # per-keytile V_aug block: [vA(64) | 1 | vB(64) | 1]

DEBUG_OUTPUTS = False


def _round_fp32r(x: np.ndarray) -> np.ndarray:
    """Round fp32 to fp32r (11-bit mantissa, RNE)."""
    u = np.ascontiguousarray(x, dtype=np.float32).view(np.uint32)
    lsb = (u >> np.uint32(12)) & np.uint32(1)
    r = (u + np.uint32(0x7FF) + lsb) & np.uint32(0xFFFFF000)
    return r.view(np.float32)


# ---------------------------------------------------------------- graph
def build():
    nc = bacc.Bacc("TRN2", target_bir_lowering=False, debug=False,
                   num_devices=NCORE)

    # ---- DRAM parameters
    xT_d = nc.dram_tensor("xT", [DIM, R], F32R, kind="ExternalInput")
    wqkv_d = nc.dram_tensor("wqkv", [DIM, 3 * 128], F32R, kind="ExternalInput")
    bqkv_d = nc.dram_tensor("bqkv", [3, 128, 1], F32, kind="ExternalInput")
    # stats lhsT per chunk: [:, 0] x-sums cols {2jj+h}, [:, 1] sq-sums
    # cols {32+2jj+h}; both accumulate into one [40, 512] psum bank.
    onesblk_d = nc.dram_tensor("onesblk", [RC, 2, 128, 40], F32R,
                               kind="ExternalInput")
    wbln_d = nc.dram_tensor("wbln", [4, 128, 1], F32, kind="ExternalInput")
    cos_d = nc.dram_tensor("cosr", [128, R], F32, kind="ExternalInput")
    sinm_d = nc.dram_tensor("sinm", [128, R], F32, kind="ExternalInput")
    ident_d = nc.dram_tensor("ident", [128, 128], F32, kind="ExternalInput")
    ones_d = nc.dram_tensor("ones64", [128, 4 * NKT], F32R,
                            kind="ExternalInput")
    wo_d = nc.dram_tensor("wo", [DIM, DIM], BF16, kind="ExternalInput")
    borep_d = nc.dram_tensor("borep", [128, DIM], F32, kind="ExternalInput")
    out_d = nc.dram_tensor("out", [R // NCORE, DIM], F32, kind="ExternalOutput")
    if DEBUG_OUTPUTS:
        dbg_qrot = nc.dram_tensor("dbg_qrot", [128, R], F32,
                                  kind="ExternalOutput")
        dbg_krot = nc.dram_tensor("dbg_krot", [128, R], F32,
                                  kind="ExternalOutput")
        dbg_den = nc.dram_tensor("dbg_den", [16, 512], F32,
                                 kind="ExternalOutput")
        dbg_ctxn = nc.dram_tensor("dbg_ctxn", [128, R], F32,
                                  kind="ExternalOutput")

    with tile.TileContext(nc) as tc:
        with (
            tc.tile_pool(name="const", bufs=1) as cpool,
            tc.tile_pool(name="persist", bufs=1) as ppool,
            tc.tile_pool(name="chp", bufs=2) as chpool,
            tc.tile_pool(name="statp", bufs=8) as statpool,
            tc.tile_pool(name="stagp", bufs=4) as stagpool,
            tc.tile_pool(name="dram", bufs=1, space="DRAM") as dpool,
        ):
            # ---- constants in SBUF
            wqkv_sb = cpool.tile([128, KT_DIM, 384], F32R)
            nc.sync.dma_start(
                out=wqkv_sb[:],
                in_=wqkv_d.ap().rearrange("(k p) c -> p k c", p=128))
            bq_sb = cpool.tile([128, 1], F32)
            bk_sb = cpool.tile([128, 1], F32)
            bv_sb = cpool.tile([128, 1], F32)
            nc.sync.dma_start(out=bq_sb[:], in_=bqkv_d.ap()[0])
            nc.sync.dma_start(out=bk_sb[:], in_=bqkv_d.ap()[1])
            nc.sync.dma_start(out=bv_sb[:], in_=bqkv_d.ap()[2])
            onesblk_sb = cpool.tile([128, RC, 2, 40], F32R)
            nc.sync.dma_start(
                out=onesblk_sb[:],
                in_=onesblk_d.ap().rearrange("j s p c -> p j s c"))
            wlnq_sb = cpool.tile([128, 1], F32)
            blnq_sb = cpool.tile([128, 1], F32)
            wlnk_sb = cpool.tile([128, 1], F32)
            blnk_sb = cpool.tile([128, 1], F32)
            nc.sync.dma_start(out=wlnq_sb[:], in_=wbln_d.ap()[0])
            nc.sync.dma_start(out=blnq_sb[:], in_=wbln_d.ap()[1])
            nc.sync.dma_start(out=wlnk_sb[:], in_=wbln_d.ap()[2])
            nc.sync.dma_start(out=blnk_sb[:], in_=wbln_d.ap()[3])
            ident_sb = cpool.tile([128, 128], F32)
            nc.sync.dma_start(out=ident_sb[:], in_=ident_d.ap()[:, :])
            borep_sb = cpool.tile([128, DIM], F32)
            nc.sync.dma_start(out=borep_sb[:], in_=borep_d.ap()[:, :])

            # ---- persistent tensors (batch-split Q/K; in-place LN+RoPE)
            qkt = {}
            for g in range(B):
                qkt[("q", g)] = ppool.tile([128, N], F32R, tag=f"q{g}",
                                           name=f"qraw{g}")
                qkt[("k", g)] = ppool.tile([128, N], F32R, tag=f"k{g}",
                                           name=f"kraw{g}")
            vaug = ppool.tile([128, 2 * NKT * VSTRIDE], F32R, tag="vaug")
            ctxn_a = ppool.tile([64, R], BF16, tag="ctxn_a")
            ctxn_b = ppool.tile([64, R], BF16, tag="ctxn_b")

            nc.gpsimd.dma_start(
                out=vaug[:].rearrange("p (k c) -> p k c", c=65)[:, :, 64:65],
                in_=ones_d.ap()[:, :])

            stat_dr = {}
            den_dr = dpool.tile([16, 512], BF16, name="den_dr")
            rec_dr = dpool.tile([16, 512], F32, name="rec_dr")
            denpacks = []

            # ---------------- emission helpers ----------------
            def emit_proj_row(r, xtpool, vchpool, ps1, ps1v, statps):
                """Project row-chunk r for q, k, v (+ inline stats MMs)."""
                g, jj = r // 4, r % 4
                xts = []
                for kt in range(KT_DIM):
                    xt = xtpool.tile([128, 512], F32R, tag="xt",
                                     name=f"xt_{r}_{kt}")
                    nc.sync.dma_start(
                        out=xt[:],
                        in_=xT_d.ap()[kt * 128:(kt + 1) * 128,
                                      r * 512:(r + 1) * 512])
                    xts.append(xt)
                for m, name, bias in ((0, "q", bq_sb), (1, "k", bk_sb),
                                      (2, "v", bv_sb)):
                    ps = ps1.tile([128, 512], F32, tag="proj",
                                  name=f"proj_{m}_{r}")
                    for kt in range(KT_DIM):
                        nc.tensor.matmul(
                            ps[:], wqkv_sb[:, kt, m * 128:(m + 1) * 128],
                            xts[kt][:],
                            start=(kt == 0), stop=(kt == KT_DIM - 1))
                    if m < 2:
                        dest = qkt[(name, g)]
                        nc.vector.tensor_scalar(
                            dest[:, jj * 512:(jj + 1) * 512], ps[:],
                            bias[:], None, ADD)
                        sps = statps[(name, g)]
                        nc.tensor.matmul(
                            sps[:], onesblk_sb[:, r, 0, :],
                            dest[:, jj * 512:(jj + 1) * 512],
                            start=(jj == 0), stop=False)
                        sqc = chpool.tile([128, 512], F32R, tag="sqc",
                                          name=f"sqc_{name}_{r}")
                        nc.vector.tensor_tensor(
                            sqc[:],
                            dest[:, jj * 512:(jj + 1) * 512].bitcast(F32),
                            dest[:, jj * 512:(jj + 1) * 512].bitcast(F32),
                            MUL)
                        nc.tensor.matmul(
                            sps[:], onesblk_sb[:, r, 1, :], sqc[:],
                            start=False, stop=(jj == 3))
                    else:
                        vch = vchpool.tile([128, 512], F32, tag="vch",
                                           name=f"vch_{r}")
                        nc.vector.tensor_scalar(vch[:], ps[:], bias[:],
                                                None, ADD)
                        for sseg in range(4):
                            kt_glob = r * 4 + sseg
                            tps = ps1v.tile([128, 128], F32, tag="vtr",
                                            name=f"vtr_{kt_glob}")
                            nc.tensor.transpose(
                                tps[:], vch[:, sseg * 128:(sseg + 1) * 128],
                                ident_sb[:])
                            vb = kt_glob * VSTRIDE
                            nc.vector.tensor_copy(
                                vaug[:, vb:vb + 64], tps[:, 0:64])
                            nc.vector.tensor_copy(
                                vaug[:, vb + 65:vb + 129], tps[:, 64:128])

            def emit_statmath(name, g, statps):
                """stat bank [40, 512]: rows 0-7 x-sums, 32-39 sq-sums ->
                rstd/murstd [8, 512] staged to DRAM."""
                sps = statps[(name, g)]
                mu = statpool.tile([8, 512], F32, tag="stat_sb",
                                   name=f"mu_{name}{g}")
                msqe = statpool.tile([8, 512], F32, tag="stat_sb",
                                     name=f"msqe_{name}{g}")
                nc.vector.tensor_scalar(mu[:], sps[0:8, :], 1.0 / HD,
                                        None, MUL)
                nc.vector.tensor_scalar(msqe[:], sps[32:40, :], 1.0 / HD,
                                        EPS, MUL, ADD)
                var = statpool.tile([8, 512], F32, tag="stat_sb",
                                    name=f"var_{name}{g}")
                nc.vector.tensor_tensor(var[:], mu[:], mu[:], MUL)
                nc.vector.tensor_tensor(var[:], msqe[:], var[:], SUB)
                sd = statpool.tile([8, 512], F32, tag="stat_sb",
                                   name=f"sd_{name}{g}")
                nc.scalar.activation(sd[:], var[:],
                                     mybir.ActivationFunctionType.Sqrt)
                rstd = statpool.tile([8, 512], F32, tag="stat_sb",
                                     name=f"rstd_{name}{g}")
                nc.vector.reciprocal(rstd[:], sd[:])
                murstd = statpool.tile([8, 512], F32, tag="stat_sb",
                                       name=f"murstd_{name}{g}")
                nc.vector.tensor_tensor(murstd[:], mu[:], rstd[:], MUL)
                rdr = dpool.tile([8, 512], F32, name=f"rstd_dr_{name}{g}")
                mdr = dpool.tile([8, 512], F32, name=f"mur_dr_{name}{g}")
                nc.scalar.dma_start(out=rdr[:], in_=rstd[:])
                nc.scalar.dma_start(out=mdr[:], in_=murstd[:])
                stat_dr[(name, g)] = (rdr, mdr)

            def emit_apply(name, g, jj, w_sb, b_sb):
                """LN apply + RoPE for chunk jj of batch g (in place)."""
                traw = qkt[(name, g)]
                rdr, mdr = stat_dr[(name, g)]
                jsl = slice(jj * 512, (jj + 1) * 512)
                gsl = slice(g * N + jj * 512, g * N + (jj + 1) * 512)
                cosc = chpool.tile([128, 512], F32, tag="cosc",
                                   name=f"cosc_{name}_{g}{jj}")
                sinc = chpool.tile([128, 512], F32, tag="sinc",
                                   name=f"sinc_{name}_{g}{jj}")
                nc.scalar.dma_start(out=cosc[:], in_=cos_d.ap()[:, gsl])
                nc.scalar.dma_start(out=sinc[:], in_=sinm_d.ap()[:, gsl])
                rep_r = chpool.tile([128, 512], F32, tag="rep_r",
                                    name=f"rep_r_{name}_{g}{jj}")
                rep_m = chpool.tile([128, 512], F32, tag="rep_m",
                                    name=f"rep_m_{name}_{g}{jj}")
                for h in range(2):
                    stg_r = stagpool.tile([1, 512], F32, tag="stg",
                                          name=f"sr_{name}_{g}{jj}_{h}")
                    stg_m = stagpool.tile([1, 512], F32, tag="stg",
                                          name=f"sm_{name}_{g}{jj}_{h}")
                    nc.scalar.dma_start(out=stg_r[:], in_=rdr[2 * jj + h])
                    nc.scalar.dma_start(out=stg_m[:], in_=mdr[2 * jj + h])
                    if h == 0:
                        nc.gpsimd.partition_broadcast(
                            rep_r[0:64, :], stg_r[:], channels=64)
                        nc.gpsimd.partition_broadcast(
                            rep_m[0:64, :], stg_m[:], channels=64)
                    else:
                        tmp_r = stagpool.tile([64, 512], F32, tag="tmpb",
                                              name=f"tr_{name}_{g}{jj}")
                        tmp_m = stagpool.tile([64, 512], F32, tag="tmpb",
                                              name=f"tm_{name}_{g}{jj}")
                        nc.gpsimd.partition_broadcast(
                            tmp_r[:], stg_r[:], channels=64)
                        nc.gpsimd.partition_broadcast(
                            tmp_m[:], stg_m[:], channels=64)
                        nc.scalar.dma_start(out=rep_r[64:128, :],
                                            in_=tmp_r[:])
                        nc.scalar.dma_start(out=rep_m[64:128, :],
                                            in_=tmp_m[:])
                tn = chpool.tile([128, 512], F32, tag="tn",
                                 name=f"tn_{name}_{g}{jj}")
                nc.vector.tensor_tensor(tn[:], traw[:, jsl].bitcast(F32),
                                        rep_r[:], MUL)
                nc.vector.tensor_tensor(tn[:], tn[:], rep_m[:], SUB)
                nc.vector.tensor_scalar(tn[:], tn[:], w_sb[:], b_sb[:],
                                        MUL, ADD)
                swp = chpool.tile([128, 512], F32, tag="swp",
                                  name=f"swp_{name}_{g}{jj}")
                for (dst, src) in ((0, 32), (32, 0), (64, 96), (96, 64)):
                    nc.scalar.dma_start(out=swp[dst:dst + 32, :],
                                        in_=tn[src:src + 32, :])
                t1 = chpool.tile([128, 512], F32, tag="t1",
                                 name=f"t1_{name}_{g}{jj}")
                nc.vector.tensor_tensor(t1[:], tn[:], cosc[:], MUL)
                nc.vector.tensor_tensor(swp[:], swp[:], sinc[:], MUL)
                nc.vector.tensor_tensor(traw[:, jsl], t1[:], swp[:], ADD)

            def emit_sdpa_qc(g, qc, exppool, ctxupool, ps_sc, ps_ctx,
                             ctxu_tiles):
                qrot = qkt[("q", g)]
                krot = qkt[("k", g)]
                qsl = slice(qc * 512, (qc + 1) * 512)
                ctxps_a = ps_ctx.tile([65, 512], F32, tag="ctx",
                                      name=f"ctxa_{g}{qc}")
                ctxps_b = ps_ctx.tile([65, 512], F32, tag="ctx",
                                      name=f"ctxb_{g}{qc}")
                for kt in range(NKT):
                    ksl = slice(kt * 128, (kt + 1) * 128)
                    scps = ps_sc.tile([128, 1024], F32, tag="sc",
                                      name=f"sc_{g}{qc}{kt}")
                    nc.tensor.matmul(scps[:, 0:512], krot[0:64, ksl],
                                     qrot[0:64, qsl], start=True, stop=True,
                                     tile_position=(0, 0))
                    nc.tensor.matmul(scps[:, 512:1024], krot[64:128, ksl],
                                     qrot[64:128, qsl], start=True,
                                     stop=True, tile_position=(64, 0))
                    expt = exppool.tile([128, 1024], F32R, tag="expt",
                                        name=f"ex_{g}{qc}{kt}")
                    nc.scalar.activation(expt[:], scps[:],
                                         mybir.ActivationFunctionType.Exp,
                                         scale=float(HD) ** -0.5)
                    vbase = (g * NKT + kt) * VSTRIDE
                    nc.tensor.matmul(ctxps_a[:], vaug[:, vbase:vbase + 65],
                                     expt[:, 0:512],
                                     start=(kt == 0), stop=(kt == NKT - 1))
                    nc.tensor.matmul(ctxps_b[:],
                                     vaug[:, vbase + 65:vbase + 130],
                                     expt[:, 512:1024],
                                     start=(kt == 0), stop=(kt == NKT - 1))
                cua = ctxupool.tile([65, 512], BF16, tag="ctxu",
                                    name=f"cua_{g}{qc}")
                cub = ctxupool.tile([65, 512], BF16, tag="ctxu",
                                    name=f"cub_{g}{qc}")
                nc.vector.tensor_copy(cua[:], ctxps_a[:])
                nc.vector.tensor_copy(cub[:], ctxps_b[:])
                idx = g * 8 + qc * 2
                nc.gpsimd.dma_start(out=den_dr[idx], in_=cua[64:65, :])
                nc.gpsimd.dma_start(out=den_dr[idx + 1], in_=cub[64:65, :])
                ctxu_tiles[qc] = (cua, cub)

            def emit_normalize(g, sp3, ctxu_tiles):
                denpack = sp3.tile([8, 512], BF16, tag="denpack",
                                   name=f"dp{g}")
                nc.gpsimd.dma_start(
                    out=denpack[:], in_=den_dr.opt()[g * 8:(g + 1) * 8, :])
                recip = sp3.tile([8, 512], F32, tag="recip", name=f"rc{g}")
                nc.vector.reciprocal(recip[:], denpack[:])
                nc.gpsimd.dma_start(out=rec_dr[g * 8:(g + 1) * 8, :],
                                    in_=recip[:])
                denpacks.append(denpack)
                for qc in range(NQC):
                    cua, cub = ctxu_tiles[qc]
                    qsl = slice(g * N + qc * 512, g * N + (qc + 1) * 512)
                    for h, cu, dst in ((0, cua, ctxn_a), (1, cub, ctxn_b)):
                        stg = sp3.tile([1, 512], F32, tag="stg3",
                                       name=f"st{g}{qc}{h}")
                        nc.gpsimd.dma_start(
                            out=stg[:], in_=rec_dr[g * 8 + qc * 2 + h])
                        rep = sp3.tile([64, 512], F32, tag="rep",
                                       name=f"rp{g}{qc}{h}")
                        nc.gpsimd.partition_broadcast(rep[:], stg[:],
                                                      channels=64)
                        nc.gpsimd.tensor_tensor(dst[:, qsl], cu[0:64, :],
                                                rep[:], MUL)

            # ---------------- pipelined emission ----------------
            with (
                tc.tile_pool(name="xtp", bufs=8) as xtpool,
                tc.tile_pool(name="vchp", bufs=3) as vchpool,
                tc.tile_pool(name="ps1", bufs=3, space="PSUM") as ps1,
                tc.tile_pool(name="ps1v", bufs=1, space="PSUM") as ps1v,
                tc.tile_pool(name="ps2", bufs=4, space="PSUM") as ps2,
            ):
                statps = {}
                for tname in ("q", "k"):
                    for g in range(B):
                        statps[(tname, g)] = ps2.tile(
                            [40, 512], F32, tag="stat",
                            name=f"stat_{tname}{g}")
                for r in range(4):
                    emit_proj_row(r, xtpool, vchpool, ps1, ps1v, statps)
                emit_statmath("q", 0, statps)
                emit_statmath("k", 0, statps)
                # batch-0 LN interleaved with batch-1 projections
                for jj in range(4):
                    emit_apply("q", 0, jj, wlnq_sb, blnq_sb)
                    emit_proj_row(4 + jj, xtpool, vchpool, ps1, ps1v, statps)
                    emit_apply("k", 0, jj, wlnk_sb, blnk_sb)
                emit_statmath("q", 1, statps)
                emit_statmath("k", 1, statps)

            # batch-1 LN interleaved with batch-0 SDPA
            with (
                tc.tile_pool(name="expp", bufs=3) as exppool,
                tc.tile_pool(name="ctxup", bufs=16) as ctxupool,
                tc.tile_pool(name="sp3", bufs=2) as sp3,
                tc.tile_pool(name="ps_sc", bufs=2, space="PSUM") as ps_sc,
                tc.tile_pool(name="ps_ctx", bufs=4, space="PSUM") as ps_ctx,
            ):
                ctxu0, ctxu1 = {}, {}
                for jj in range(4):
                    emit_apply("q", 1, jj, wlnq_sb, blnq_sb)
                    emit_sdpa_qc(0, jj, exppool, ctxupool, ps_sc, ps_ctx,
                                 ctxu0)
                    emit_apply("k", 1, jj, wlnk_sb, blnk_sb)
                emit_normalize(0, sp3, ctxu0)
                for qc in range(NQC):
                    emit_sdpa_qc(1, qc, exppool, ctxupool, ps_sc, ps_ctx,
                                 ctxu1)
                emit_normalize(1, sp3, ctxu1)

            # ================= AllToAll (bf16) =================
            a2a_in = dpool.tile([NCORE, 128, 512], BF16)
            a2a_out = dpool.tile([NCORE, 128, 512], BF16)
            for j in range(NCORE):
                nc.gpsimd.dma_start(out=a2a_in[j][0:64, :],
                                    in_=ctxn_a[:, j * 512:(j + 1) * 512])
                nc.gpsimd.dma_start(out=a2a_in[j][64:128, :],
                                    in_=ctxn_b[:, j * 512:(j + 1) * 512])
            nc.gpsimd.collective_compute(
                "AllToAll", mybir.AluOpType.bypass,
                ins=[a2a_in.opt()], outs=[a2a_out.opt()],
                replica_groups=[list(range(NCORE))],
            )

            # ================= output projection (bf16) ==============
            with (
                tc.tile_pool(name="wop", bufs=3) as wopool,
                tc.tile_pool(name="sp5", bufs=4) as sp5,
                tc.tile_pool(name="ps_out", bufs=4, space="PSUM") as ps_out,
            ):
                ops = [ps_out.tile([128, 1024], F32, tag="outp",
                                   name=f"outp{i}") for i in range(4)]
                for kt in range(KT_DIM):
                    wo_sb = wopool.tile([128, DIM], BF16, tag="wo",
                                        name=f"wo{kt}")
                    nc.sync.dma_start(
                        out=wo_sb[:],
                        in_=wo_d.ap()[kt * 128:(kt + 1) * 128, :])
                    cg = wopool.tile([128, 512], BF16, tag="ctxg",
                                     name=f"cg{kt}")
                    nc.sync.dma_start(out=cg[:], in_=a2a_out[kt])
                    for rt in range(4):
                        for nh in range(2):
                            nc.tensor.matmul(
                                ops[rt][:, nh * 512:(nh + 1) * 512],
                                cg[:, rt * 128:(rt + 1) * 128],
                                wo_sb[:, nh * 512:(nh + 1) * 512],
                                start=(kt == 0), stop=(kt == KT_DIM - 1))
                for rt in range(4):
                    osb = sp5.tile([128, DIM], F32, tag="osb",
                                   name=f"osb{rt}")
                    nc.vector.tensor_tensor(osb[:], ops[rt][:], borep_sb[:],
                                            ADD)
                    nc.sync.dma_start(
                        out=out_d.ap()[rt * 128:(rt + 1) * 128, :],
                        in_=osb[:])

            if DEBUG_OUTPUTS:
                for g in range(B):
                    nc.sync.dma_start(
                        out=dbg_qrot.ap()[:, g * N:(g + 1) * N],
                        in_=qkt[("q", g)][:].bitcast(F32))
                    nc.sync.dma_start(
                        out=dbg_krot.ap()[:, g * N:(g + 1) * N],
                        in_=qkt[("k", g)][:].bitcast(F32))

            if DEBUG_OUTPUTS:
                nc.gpsimd.dma_start(out=dbg_den.ap()[:, :], in_=den_dr.opt())
                nc.gpsimd.dma_start(out=dbg_ctxn.ap()[0:64, :], in_=ctxn_a[:])
                nc.gpsimd.dma_start(out=dbg_ctxn.ap()[64:128, :],
                                    in_=ctxn_b[:])

    nc.compile()
    return nc


# ---------------------------------------------------------------- host side
def prepare_in_maps(x, rotary_cos, rotary_sin, Wq, bq, Wk, bk, Wv, bv,
                    q_norm_w, q_norm_b, k_norm_w, k_norm_b, Wo, bo):
    import ml_dtypes

    x = np.asarray(x, np.float32)
    xT = _round_fp32r(np.ascontiguousarray(x.reshape(R, DIM).T))

    Wcat = np.concatenate([np.asarray(Wq, np.float32),
                           np.asarray(Wk, np.float32),
                           np.asarray(Wv, np.float32)], axis=1)
    bcat = np.concatenate([np.asarray(bq, np.float32),
                           np.asarray(bk, np.float32),
                           np.asarray(bv, np.float32)])

    def head_cols(h, part):
        s = 192 * h + 64 * part
        return np.arange(s, s + 64)

    cos_flat = np.asarray(rotary_cos, np.float32).reshape(R, HD).T
    sin_flat = np.asarray(rotary_sin, np.float32).reshape(R, HD).T
    sinm = sin_flat.copy()
    sinm[0:32] = -sin_flat[0:32]
    cos_rep = np.ascontiguousarray(np.tile(cos_flat, (2, 1)))
    sinm_rep = np.ascontiguousarray(np.tile(sinm, (2, 1)))

    onesblk = np.zeros((RC, 2, 128, 40), np.float32)
    for j in range(RC):
        jj = j % 4
        onesblk[j, 0, 0:64, 2 * jj] = 1.0
        onesblk[j, 0, 64:128, 2 * jj + 1] = 1.0
        onesblk[j, 1, 0:64, 32 + 2 * jj] = 1.0
        onesblk[j, 1, 64:128, 32 + 2 * jj + 1] = 1.0

    wbln = np.stack([
        np.tile(np.asarray(q_norm_w, np.float32), 2)[:, None],
        np.tile(np.asarray(q_norm_b, np.float32), 2)[:, None],
        np.tile(np.asarray(k_norm_w, np.float32), 2)[:, None],
        np.tile(np.asarray(k_norm_b, np.float32), 2)[:, None],
    ])

    ident = np.eye(128, dtype=np.float32)
    ones64 = np.ones((128, 4 * NKT), np.float32)
    borep = np.tile(np.asarray(bo, np.float32)[None, :], (128, 1))
    wo_bf = np.asarray(Wo, np.float32).astype(ml_dtypes.bfloat16)

    in_maps = []
    for c in range(NCORE):
        hA, hB = 2 * c, 2 * c + 1
        cols = np.concatenate([
            head_cols(hA, 0), head_cols(hB, 0),
            head_cols(hA, 1), head_cols(hB, 1),
            head_cols(hA, 2), head_cols(hB, 2),
        ])
        wqkv_c = _round_fp32r(np.ascontiguousarray(Wcat[:, cols]))
        bqkv_c = np.ascontiguousarray(bcat[cols].reshape(3, 128, 1))
        in_maps.append({
            "xT": xT,
            "wqkv": wqkv_c,
            "bqkv": bqkv_c,
            "onesblk": onesblk,
            "wbln": wbln,
            "cosr": cos_rep,
            "sinm": sinm_rep,
            "ident": ident,
            "ones64": ones64,
            "wo": wo_bf,
            "borep": borep,
        })
    return in_maps


def assemble_output(results):
    out = np.empty((R, DIM), np.float32)
    for c in range(NCORE):
        out[c * 512:(c + 1) * 512] = results[c]["out"]
    return out.reshape(B, N, DIM)


_NC_CACHE = []


def kernel(**inputs) -> np.ndarray:
    if not _NC_CACHE:
        _NC_CACHE.append(build())
    nc = _NC_CACHE[0]
    in_maps = prepare_in_maps(**inputs)
    res = run_bass_kernel_spmd(nc, in_maps, core_ids=list(range(NCORE)))
    return assemble_output(res.results)


# revision 40
# speedup vs baseline: 1.5580x; 1.4359x over previous
"""Trainium2 Bass kernel for nn_Attention_17008070493108.

Dense transformer attention block: QKV proj -> per-head LayerNorm -> RoPE
-> SDPA -> out proj, for x[2, 2048, 1024], H=16 heads, head_dim=64.

Sharding: tensor-parallel over heads. Each of the 8 NeuronCores owns 2
heads end-to-end (QKV column slices, norm, RoPE, attention). The
per-head context vectors are exchanged with a single AllToAll so each
core finishes the output projection (contraction over the full 1024
model dims) for its own 512-row slice of the output; the host
concatenates row slices.

Layouts (per core):
  xT          [1024, 4096] model-dim on partitions (host-transposed x)
  QT/KT       [128, 2048]x2 (batch-split) heads stacked on partitions
  scoresT     [128 keys, q] key tiles on partitions; softmax denominator
                          via a ones-column appended to V (ctx_aug row 64)
  ctx         [64, 4096]x2 -> AllToAll (bf16) -> out rows [512, 1024]

The emission is software-pipelined: LayerNorm+RoPE of batch 0 overlaps
the batch-1 projections on PE, and SDPA of batch 0 overlaps the batch-1
LayerNorm on DVE. Matmuls run in float32r (fp32 with 11-bit mantissa,
full PE rate); the output projection runs in bf16.
"""

import numpy as np

from concourse import bacc, tile, mybir
from concourse.bass_utils import run_bass_kernel_spmd

# ---------------------------------------------------------------- constants
DIM = 1024
H = 16
HD = 64
B = 2
N = 2048
R = B * N          # 4096 flattened rows
NCORE = 8
EPS = 1e-6

F32 = mybir.dt.float32
F32R = mybir.dt.float32r
BF16 = mybir.dt.bfloat16
ADD = mybir.AluOpType.add
SUB = mybir.AluOpType.subtract
MUL = mybir.AluOpType.mult

RC = R // 512        # 8 row chunks of 512
KT_DIM = DIM // 128  # 8 contraction tiles for the projections
NQC = N // 512       # 4 q chunks per batch
NKT = N // 128       # 16 key tiles per batch
VSTRIDE = 130        # per-keytile V_aug block: [vA(64) | 1 | vB(64) | 1]

DEBUG_OUTPUTS = False


def _round_fp32r(x: np.ndarray) -> np.ndarray:
    """Round fp32 to fp32r (11-bit mantissa, RNE)."""
    u = np.ascontiguousarray(x, dtype=np.float32).view(np.uint32)
    lsb = (u >> np.uint32(12)) & np.uint32(1)
    r = (u + np.uint32(0x7FF) + lsb) & np.uint32(0xFFFFF000)
    return r.view(np.float32)


# ---------------------------------------------------------------- graph
def build():
    nc = bacc.Bacc("TRN2", target_bir_lowering=False, debug=False,
                   num_devices=NCORE)

    # ---- DRAM parameters
    xT_d = nc.dram_tensor("xT", [DIM, R], F32R, kind="ExternalInput")
    wqkv_d = nc.dram_tensor("wqkv", [DIM, 3 * 128], F32R, kind="ExternalInput")
    bqkv_d = nc.dram_tensor("bqkv", [3, 128, 1], F32, kind="ExternalInput")
    # stats lhsT per chunk: [:, 0] x-sums cols {2jj+h}, [:, 1] sq-sums
    # cols {32+2jj+h}; both accumulate into one [40, 512] psum bank.
    onesblk_d = nc.dram_tensor("onesblk", [RC, 2, 128, 40], F32R,
                               kind="ExternalInput")
    wbln_d = nc.dram_tensor("wbln", [4, 128, 1], F32, kind="ExternalInput")
    cos_d = nc.dram_tensor("cosr", [128, R], F32, kind="ExternalInput")
    sinm_d = nc.dram_tensor("sinm", [128, R], F32, kind="ExternalInput")
    ident_d = nc.dram_tensor("ident", [128, 128], F32, kind="ExternalInput")
    ones_d = nc.dram_tensor("ones64", [128, 4 * NKT], F32R,
                            kind="ExternalInput")
    wo_d = nc.dram_tensor("wo", [DIM, DIM], BF16, kind="ExternalInput")
    borep_d = nc.dram_tensor("borep", [128, DIM], F32, kind="ExternalInput")
    out_d = nc.dram_tensor("out", [R // NCORE, DIM], F32, kind="ExternalOutput")
    if DEBUG_OUTPUTS:
        dbg_qrot = nc.dram_tensor("dbg_qrot", [128, R], F32,
                                  kind="ExternalOutput")
        dbg_krot = nc.dram_tensor("dbg_krot", [128, R], F32,
                                  kind="ExternalOutput")
        dbg_den = nc.dram_tensor("dbg_den", [16, 512], F32,
                                 kind="ExternalOutput")
        dbg_ctxn = nc.dram_tensor("dbg_ctxn", [128, R], F32,
                                  kind="ExternalOutput")

    with tile.TileContext(nc) as tc:
        with (
            tc.tile_pool(name="const", bufs=1) as cpool,
            tc.tile_pool(name="persist", bufs=1) as ppool,
            tc.tile_pool(name="chp", bufs=2) as chpool,
            tc.tile_pool(name="statp", bufs=8) as statpool,
            tc.tile_pool(name="stagp", bufs=4) as stagpool,
            tc.tile_pool(name="dram", bufs=1, space="DRAM") as dpool,
        ):
            # ---- constants in SBUF
            wqkv_sb = cpool.tile([128, KT_DIM, 384], F32R)
            nc.sync.dma_start(
                out=wqkv_sb[:],
                in_=wqkv_d.ap().rearrange("(k p) c -> p k c", p=128))
            bq_sb = cpool.tile([128, 1], F32)
            bk_sb = cpool.tile([128, 1], F32)
            bv_sb = cpool.tile([128, 1], F32)
            nc.sync.dma_start(out=bq_sb[:], in_=bqkv_d.ap()[0])
            nc.sync.dma_start(out=bk_sb[:], in_=bqkv_d.ap()[1])
            nc.sync.dma_start(out=bv_sb[:], in_=bqkv_d.ap()[2])
            onesblk_sb = cpool.tile([128, RC, 2, 40], F32R)
            nc.sync.dma_start(
                out=onesblk_sb[:],
                in_=onesblk_d.ap().rearrange("j s p c -> p j s c"))
            wlnq_sb = cpool.tile([128, 1], F32)
            blnq_sb = cpool.tile([128, 1], F32)
            wlnk_sb = cpool.tile([128, 1], F32)
            blnk_sb = cpool.tile([128, 1], F32)
            nc.sync.dma_start(out=wlnq_sb[:], in_=wbln_d.ap()[0])
            nc.sync.dma_start(out=blnq_sb[:], in_=wbln_d.ap()[1])
            nc.sync.dma_start(out=wlnk_sb[:], in_=wbln_d.ap()[2])
            nc.sync.dma_start(out=blnk_sb[:], in_=wbln_d.ap()[3])
            ident_sb = cpool.tile([128, 128], F32)
            nc.sync.dma_start(out=ident_sb[:], in_=ident_d.ap()[:, :])
            borep_sb = cpool.tile([128, DIM], F32)
            nc.sync.dma_start(out=borep_sb[:], in_=borep_d.ap()[:, :])

            # ---- persistent tensors (batch-split Q/K; in-place LN+RoPE)
            qkt = {}
            for g in range(B):
                qkt[("q", g)] = ppool.tile([128, N], F32R, tag=f"q{g}",
                                           name=f"qraw{g}")
                qkt[("k", g)] = ppool.tile([128, N], F32R, tag=f"k{g}",
                                           name=f"kraw{g}")
            vaug = ppool.tile([128, 2 * NKT * VSTRIDE], F32R, tag="vaug")
            ctxn_a = ppool.tile([64, R], BF16, tag="ctxn_a")
            ctxn_b = ppool.tile([64, R], BF16, tag="ctxn_b")

            nc.gpsimd.dma_start(
                out=vaug[:].rearrange("p (k c) -> p k c", c=65)[:, :, 64:65],
                in_=ones_d.ap()[:, :])

            stat_dr = {}
            den_dr = dpool.tile([16, 512], BF16, name="den_dr")
            rec_dr = dpool.tile([16, 512], F32, name="rec_dr")
            denpacks = []

            # ---------------- emission helpers ----------------
            def emit_proj_row(r, xtpool, vchpool, ps1, ps1v, statps):
                """Project row-chunk r for q, k, v (+ inline stats MMs)."""
                g, jj = r // 4, r % 4
                xts = []
                for kt in range(KT_DIM):
                    xt = xtpool.tile([128, 512], F32R, tag="xt",
                                     name=f"xt_{r}_{kt}")
                    nc.sync.dma_start(
                        out=xt[:],
                        in_=xT_d.ap()[kt * 128:(kt + 1) * 128,
                                      r * 512:(r + 1) * 512])
                    xts.append(xt)
                for m, name, bias in ((0, "q", bq_sb), (1, "k", bk_sb),
                                      (2, "v", bv_sb)):
                    ps = ps1.tile([128, 512], F32, tag="proj",
                                  name=f"proj_{m}_{r}")
                    for kt in range(KT_DIM):
                        nc.tensor.matmul(
                            ps[:], wqkv_sb[:, kt, m * 128:(m + 1) * 128],
                            xts[kt][:],
                            start=(kt == 0), stop=(kt == KT_DIM - 1))
                    if m < 2:
                        dest = qkt[(name, g)]
                        nc.vector.tensor_scalar(
                            dest[:, jj * 512:(jj + 1) * 512], ps[:],
                            bias[:], None, ADD)
                        sps = statps[(name, g)]
                        nc.tensor.matmul(
                            sps[:], onesblk_sb[:, r, 0, :],
                            dest[:, jj * 512:(jj + 1) * 512],
                            start=(jj == 0), stop=False)
                        sqc = chpool.tile([128, 512], F32R, tag="sqc",
                                          name=f"sqc_{name}_{r}")
                        nc.vector.tensor_tensor(
                            sqc[:],
                            dest[:, jj * 512:(jj + 1) * 512].bitcast(F32),
                            dest[:, jj * 512:(jj + 1) * 512].bitcast(F32),
                            MUL)
                        nc.tensor.matmul(
                            sps[:], onesblk_sb[:, r, 1, :], sqc[:],
                            start=False, stop=(jj == 3))
                    else:
                        vch = vchpool.tile([128, 512], F32, tag="vch",
                                           name=f"vch_{r}")
                        nc.vector.tensor_scalar(vch[:], ps[:], bias[:],
                                                None, ADD)
                        for sseg in range(4):
                            kt_glob = r * 4 + sseg
                            tps = ps1v.tile([128, 128], F32, tag="vtr",
                                            name=f"vtr_{kt_glob}")
                            nc.tensor.transpose(
                                tps[:], vch[:, sseg * 128:(sseg + 1) * 128],
                                ident_sb[:])
                            vb = kt_glob * VSTRIDE
                            nc.vector.tensor_copy(
                                vaug[:, vb:vb + 64], tps[:, 0:64])
                            nc.vector.tensor_copy(
                                vaug[:, vb + 65:vb + 129], tps[:, 64:128])

            def emit_statmath(name, g, statps):
                """stat bank [40, 512]: rows 0-7 x-sums, 32-39 sq-sums ->
                rstd/murstd [8, 512] staged to DRAM."""
                sps = statps[(name, g)]
                mu = statpool.tile([8, 512], F32, tag="stat_sb",
                                   name=f"mu_{name}{g}")
                msqe = statpool.tile([8, 512], F32, tag="stat_sb",
                                     name=f"msqe_{name}{g}")
                nc.vector.tensor_scalar(mu[:], sps[0:8, :], 1.0 / HD,
                                        None, MUL)
                nc.vector.tensor_scalar(msqe[:], sps[32:40, :], 1.0 / HD,
                                        EPS, MUL, ADD)
                var = statpool.tile([8, 512], F32, tag="stat_sb",
                                    name=f"var_{name}{g}")
                nc.vector.tensor_tensor(var[:], mu[:], mu[:], MUL)
                nc.vector.tensor_tensor(var[:], msqe[:], var[:], SUB)
                sd = statpool.tile([8, 512], F32, tag="stat_sb",
                                   name=f"sd_{name}{g}")
                nc.scalar.activation(sd[:], var[:],
                                     mybir.ActivationFunctionType.Sqrt)
                rstd = statpool.tile([8, 512], F32, tag="stat_sb",
                                     name=f"rstd_{name}{g}")
                nc.vector.reciprocal(rstd[:], sd[:])
                murstd = statpool.tile([8, 512], F32, tag="stat_sb",
                                       name=f"murstd_{name}{g}")
                nc.vector.tensor_tensor(murstd[:], mu[:], rstd[:], MUL)
                rdr = dpool.tile([8, 512], F32, name=f"rstd_dr_{name}{g}")
                mdr = dpool.tile([8, 512], F32, name=f"mur_dr_{name}{g}")
                nc.sync.dma_start(out=rdr[:], in_=rstd[:])
                nc.sync.dma_start(out=mdr[:], in_=murstd[:])
                stat_dr[(name, g)] = (rdr, mdr)

            def emit_apply(name, g, jj, w_sb, b_sb):
                """LN apply + RoPE for chunk jj of batch g (in place)."""
                traw = qkt[(name, g)]
                rdr, mdr = stat_dr[(name, g)]
                jsl = slice(jj * 512, (jj + 1) * 512)
                gsl = slice(g * N + jj * 512, g * N + (jj + 1) * 512)
                cosc = chpool.tile([128, 512], F32, tag="cosc",
                                   name=f"cosc_{name}_{g}{jj}")
                sinc = chpool.tile([128, 512], F32, tag="sinc",
                                   name=f"sinc_{name}_{g}{jj}")
                nc.sync.dma_start(out=cosc[:], in_=cos_d.ap()[:, gsl])
                nc.sync.dma_start(out=sinc[:], in_=sinm_d.ap()[:, gsl])
                rep_r = chpool.tile([128, 512], F32, tag="rep_r",
                                    name=f"rep_r_{name}_{g}{jj}")
                rep_m = chpool.tile([128, 512], F32, tag="rep_m",
                                    name=f"rep_m_{name}_{g}{jj}")
                for h in range(2):
                    stg_r = stagpool.tile([1, 512], F32, tag="stg",
                                          name=f"sr_{name}_{g}{jj}_{h}")
                    stg_m = stagpool.tile([1, 512], F32, tag="stg",
                                          name=f"sm_{name}_{g}{jj}_{h}")
                    nc.sync.dma_start(out=stg_r[:], in_=rdr[2 * jj + h])
                    nc.sync.dma_start(out=stg_m[:], in_=mdr[2 * jj + h])
                    if h == 0:
                        nc.gpsimd.partition_broadcast(
                            rep_r[0:64, :], stg_r[:], channels=64)
                        nc.gpsimd.partition_broadcast(
                            rep_m[0:64, :], stg_m[:], channels=64)
                    else:
                        tmp_r = stagpool.tile([64, 512], F32, tag="tmpb",
                                              name=f"tr_{name}_{g}{jj}")
                        tmp_m = stagpool.tile([64, 512], F32, tag="tmpb",
                                              name=f"tm_{name}_{g}{jj}")
                        nc.gpsimd.partition_broadcast(
                            tmp_r[:], stg_r[:], channels=64)
                        nc.gpsimd.partition_broadcast(
                            tmp_m[:], stg_m[:], channels=64)
                        nc.sync.dma_start(out=rep_r[64:128, :],
                                          in_=tmp_r[:])
                        nc.sync.dma_start(out=rep_m[64:128, :],
                                          in_=tmp_m[:])
                tn = chpool.tile([128, 512], F32, tag="tn",
                                 name=f"tn_{name}_{g}{jj}")
                nc.vector.tensor_tensor(tn[:], traw[:, jsl].bitcast(F32),
                                        rep_r[:], MUL)
                nc.vector.tensor_tensor(tn[:], tn[:], rep_m[:], SUB)
                nc.vector.tensor_scalar(tn[:], tn[:], w_sb[:], b_sb[:],
                                        MUL, ADD)
                swp = chpool.tile([128, 512], F32, tag="swp",
                                  name=f"swp_{name}_{g}{jj}")
                for (dst, src) in ((0, 32), (32, 0), (64, 96), (96, 64)):
                    nc.sync.dma_start(out=swp[dst:dst + 32, :],
                                      in_=tn[src:src + 32, :])
                t1 = chpool.tile([128, 512], F32, tag="t1",
                                 name=f"t1_{name}_{g}{jj}")
                nc.vector.tensor_tensor(t1[:], tn[:], cosc[:], MUL)
                nc.vector.tensor_tensor(swp[:], swp[:], sinc[:], MUL)
                nc.vector.tensor_tensor(traw[:, jsl], t1[:], swp[:], ADD)

            def emit_sdpa_qc(g, qc, exppool, ctxupool, ps_sc, ps_ctx,
                             ctxu_tiles):
                qrot = qkt[("q", g)]
                krot = qkt[("k", g)]
                qsl = slice(qc * 512, (qc + 1) * 512)
                ctxps_a = ps_ctx.tile([65, 512], F32, tag="ctx",
                                      name=f"ctxa_{g}{qc}")
                ctxps_b = ps_ctx.tile([65, 512], F32, tag="ctx",
                                      name=f"ctxb_{g}{qc}")
                for kt in range(NKT):
                    ksl = slice(kt * 128, (kt + 1) * 128)
                    scps = ps_sc.tile([128, 1024], F32, tag="sc",
                                      name=f"sc_{g}{qc}{kt}")
                    nc.tensor.matmul(scps[:, 0:512], krot[0:64, ksl],
                                     qrot[0:64, qsl], start=True, stop=True,
                                     tile_position=(0, 0))
                    nc.tensor.matmul(scps[:, 512:1024], krot[64:128, ksl],
                                     qrot[64:128, qsl], start=True,
                                     stop=True, tile_position=(64, 0))
                    expt = exppool.tile([128, 1024], F32R, tag="expt",
                                        name=f"ex_{g}{qc}{kt}")
                    nc.scalar.activation(expt[:], scps[:],
                                         mybir.ActivationFunctionType.Exp,
                                         scale=float(HD) ** -0.5)
                    vbase = (g * NKT + kt) * VSTRIDE
                    nc.tensor.matmul(ctxps_a[:], vaug[:, vbase:vbase + 65],
                                     expt[:, 0:512],
                                     start=(kt == 0), stop=(kt == NKT - 1))
                    nc.tensor.matmul(ctxps_b[:],
                                     vaug[:, vbase + 65:vbase + 130],
                                     expt[:, 512:1024],
                                     start=(kt == 0), stop=(kt == NKT - 1))
                cua = ctxupool.tile([65, 512], BF16, tag="ctxu",
                                    name=f"cua_{g}{qc}")
                cub = ctxupool.tile([65, 512], BF16, tag="ctxu",
                                    name=f"cub_{g}{qc}")
                nc.vector.tensor_copy(cua[:], ctxps_a[:])
                nc.vector.tensor_copy(cub[:], ctxps_b[:])
                idx = g * 8 + qc * 2
                nc.gpsimd.dma_start(out=den_dr[idx], in_=cua[64:65, :])
                nc.gpsimd.dma_start(out=den_dr[idx + 1], in_=cub[64:65, :])
                ctxu_tiles[qc] = (cua, cub)

            def emit_normalize(g, sp3, ctxu_tiles):
                denpack = sp3.tile([8, 512], BF16, tag="denpack",
                                   name=f"dp{g}")
                nc.sync.dma_start(
                    out=denpack[:], in_=den_dr.opt()[g * 8:(g + 1) * 8, :])
                recip = sp3.tile([8, 512], F32, tag="recip", name=f"rc{g}")
                nc.vector.reciprocal(recip[:], denpack[:])
                nc.sync.dma_start(out=rec_dr[g * 8:(g + 1) * 8, :],
                                  in_=recip[:])
                denpacks.append(denpack)
                for qc in range(NQC):
                    cua, cub = ctxu_tiles[qc]
                    qsl = slice(g * N + qc * 512, g * N + (qc + 1) * 512)
                    for h, cu, dst in ((0, cua, ctxn_a), (1, cub, ctxn_b)):
                        stg = sp3.tile([1, 512], F32, tag="stg3",
                                       name=f"st{g}{qc}{h}")
                        nc.sync.dma_start(
                            out=stg[:], in_=rec_dr[g * 8 + qc * 2 + h])
                        rep = sp3.tile([64, 512], F32, tag="rep",
                                       name=f"rp{g}{qc}{h}")
                        nc.gpsimd.partition_broadcast(rep[:], stg[:],
                                                      channels=64)
                        nc.vector.tensor_tensor(dst[:, qsl], cu[0:64, :],
                                                rep[:], MUL)

            # ---------------- pipelined emission ----------------
            with (
                tc.tile_pool(name="xtp", bufs=8) as xtpool,
                tc.tile_pool(name="vchp", bufs=3) as vchpool,
                tc.tile_pool(name="ps1", bufs=3, space="PSUM") as ps1,
                tc.tile_pool(name="ps1v", bufs=1, space="PSUM") as ps1v,
                tc.tile_pool(name="ps2", bufs=4, space="PSUM") as ps2,
            ):
                statps = {}
                for tname in ("q", "k"):
                    for g in range(B):
                        statps[(tname, g)] = ps2.tile(
                            [40, 512], F32, tag="stat",
                            name=f"stat_{tname}{g}")
                for r in range(4):
                    emit_proj_row(r, xtpool, vchpool, ps1, ps1v, statps)
                emit_statmath("q", 0, statps)
                emit_statmath("k", 0, statps)
                # batch-0 LN interleaved with batch-1 projections
                for jj in range(4):
                    emit_apply("q", 0, jj, wlnq_sb, blnq_sb)
                    emit_proj_row(4 + jj, xtpool, vchpool, ps1, ps1v, statps)
                    emit_apply("k", 0, jj, wlnk_sb, blnk_sb)
                emit_statmath("q", 1, statps)
                emit_statmath("k", 1, statps)

            # batch-1 LN interleaved with batch-0 SDPA
            with (
                tc.tile_pool(name="expp", bufs=3) as exppool,
                tc.tile_pool(name="ctxup", bufs=16) as ctxupool,
                tc.tile_pool(name="sp3", bufs=2) as sp3,
                tc.tile_pool(name="ps_sc", bufs=2, space="PSUM") as ps_sc,
                tc.tile_pool(name="ps_ctx", bufs=4, space="PSUM") as ps_ctx,
            ):
                ctxu0, ctxu1 = {}, {}
                for jj in range(4):
                    emit_apply("q", 1, jj, wlnq_sb, blnq_sb)
                    emit_sdpa_qc(0, jj, exppool, ctxupool, ps_sc, ps_ctx,
                                 ctxu0)
                    emit_apply("k", 1, jj, wlnk_sb, blnk_sb)
                emit_normalize(0, sp3, ctxu0)
                for qc in range(NQC):
                    emit_sdpa_qc(1, qc, exppool, ctxupool, ps_sc, ps_ctx,
                                 ctxu1)
                emit_normalize(1, sp3, ctxu1)

            # ================= AllToAll (bf16) =================
            a2a_in = dpool.tile([NCORE, 128, 512], BF16)
            a2a_out = dpool.tile([NCORE, 128, 512], BF16)
            for j in range(NCORE):
                nc.gpsimd.dma_start(out=a2a_in[j][0:64, :],
                                    in_=ctxn_a[:, j * 512:(j + 1) * 512])
                nc.gpsimd.dma_start(out=a2a_in[j][64:128, :],
                                    in_=ctxn_b[:, j * 512:(j + 1) * 512])
            nc.gpsimd.collective_compute(
                "AllToAll", mybir.AluOpType.bypass,
                ins=[a2a_in.opt()], outs=[a2a_out.opt()],
                replica_groups=[list(range(NCORE))],
            )

            # ================= output projection (bf16) ==============
            with (
                tc.tile_pool(name="wop", bufs=3) as wopool,
                tc.tile_pool(name="sp5", bufs=4) as sp5,
                tc.tile_pool(name="ps_out", bufs=4, space="PSUM") as ps_out,
            ):
                ops = [ps_out.tile([128, 1024], F32, tag="outp",
                                   name=f"outp{i}") for i in range(4)]
                for kt in range(KT_DIM):
                    wo_sb = wopool.tile([128, DIM], BF16, tag="wo",
                                        name=f"wo{kt}")
                    nc.sync.dma_start(
                        out=wo_sb[:],
                        in_=wo_d.ap()[kt * 128:(kt + 1) * 128, :])
                    cg = wopool.tile([128, 512], BF16, tag="ctxg",
                                     name=f"cg{kt}")
                    nc.sync.dma_start(out=cg[:], in_=a2a_out[kt])
                    for rt in range(4):
                        for nh in range(2):
                            nc.tensor.matmul(
                                ops[rt][:, nh * 512:(nh + 1) * 512],
                                cg[:, rt * 128:(rt + 1) * 128],
                                wo_sb[:, nh * 512:(nh + 1) * 512],
                                start=(kt == 0), stop=(kt == KT_DIM - 1))
                for rt in range(4):
                    osb = sp5.tile([128, DIM], F32, tag="osb",
                                   name=f"osb{rt}")
                    nc.vector.tensor_tensor(osb[:], ops[rt][:], borep_sb[:],
                                            ADD)
                    nc.sync.dma_start(
                        out=out_d.ap()[rt * 128:(rt + 1) * 128, :],
                        in_=osb[:])

            if DEBUG_OUTPUTS:
                for g in range(B):
                    nc.sync.dma_start(
                        out=dbg_qrot.ap()[:, g * N:(g + 1) * N],
                        in_=qkt[("q", g)][:].bitcast(F32))
                    nc.sync.dma_start(
                        out=dbg_krot.ap()[:, g * N:(g + 1) * N],
                        in_=qkt[("k", g)][:].bitcast(F32))

            if DEBUG_OUTPUTS:
                nc.gpsimd.dma_start(out=dbg_den.ap()[:, :], in_=den_dr.opt())
                nc.gpsimd.dma_start(out=dbg_ctxn.ap()[0:64, :], in_=ctxn_a[:])
                nc.gpsimd.dma_start(out=dbg_ctxn.ap()[64:128, :],
                                    in_=ctxn_b[:])

    nc.compile()
    return nc


# ---------------------------------------------------------------- host side
def prepare_in_maps(x, rotary_cos, rotary_sin, Wq, bq, Wk, bk, Wv, bv,
                    q_norm_w, q_norm_b, k_norm_w, k_norm_b, Wo, bo):
    import ml_dtypes

    x = np.asarray(x, np.float32)
    xT = _round_fp32r(np.ascontiguousarray(x.reshape(R, DIM).T))

    Wcat = np.concatenate([np.asarray(Wq, np.float32),
                           np.asarray(Wk, np.float32),
                           np.asarray(Wv, np.float32)], axis=1)
    bcat = np.concatenate([np.asarray(bq, np.float32),
                           np.asarray(bk, np.float32),
                           np.asarray(bv, np.float32)])

    def head_cols(h, part):
        s = 192 * h + 64 * part
        return np.arange(s, s + 64)

    cos_flat = np.asarray(rotary_cos, np.float32).reshape(R, HD).T
    sin_flat = np.asarray(rotary_sin, np.float32).reshape(R, HD).T
    sinm = sin_flat.copy()
    sinm[0:32] = -sin_flat[0:32]
    cos_rep = np.ascontiguousarray(np.tile(cos_flat, (2, 1)))
    sinm_rep = np.ascontiguousarray(np.tile(sinm, (2, 1)))

    onesblk = np.zeros((RC, 2, 128, 40), np.float32)
    for j in range(RC):
        jj = j % 4
        onesblk[j, 0, 0:64, 2 * jj] = 1.0
        onesblk[j, 0, 64:128, 2 * jj + 1] = 1.0
        onesblk[j, 1, 0:64, 32 + 2 * jj] = 1.0
        onesblk[j, 1, 64:128, 32 + 2 * jj + 1] = 1.0

    wbln = np.stack([
        np.tile(np.asarray(q_norm_w, np.float32), 2)[:, None],
        np.tile(np.asarray(q_norm_b, np.float32), 2)[:, None],
        np.tile(np.asarray(k_norm_w, np.float32), 2)[:, None],
        np.tile(np.asarray(k_norm_b, np.float32), 2)[:, None],
    ])

    ident = np.eye(128, dtype=np.float32)
    ones64 = np.ones((128, 4 * NKT), np.float32)
    borep = np.tile(np.asarray(bo, np.float32)[None, :], (128, 1))
    wo_bf = np.asarray(Wo, np.float32).astype(ml_dtypes.bfloat16)

    in_maps = []
    for c in range(NCORE):
        hA, hB = 2 * c, 2 * c + 1
        cols = np.concatenate([
            head_cols(hA, 0), head_cols(hB, 0),
            head_cols(hA, 1), head_cols(hB, 1),
            head_cols(hA, 2), head_cols(hB, 2),
        ])
        wqkv_c = _round_fp32r(np.ascontiguousarray(Wcat[:, cols]))
        bqkv_c = np.ascontiguousarray(bcat[cols].reshape(3, 128, 1))
        in_maps.append({
            "xT": xT,
            "wqkv": wqkv_c,
            "bqkv": bqkv_c,
            "onesblk": onesblk,
            "wbln": wbln,
            "cosr": cos_rep,
            "sinm": sinm_rep,
            "ident": ident,
            "ones64": ones64,
            "wo": wo_bf,
            "borep": borep,
        })
    return in_maps


def assemble_output(results):
    out = np.empty((R, DIM), np.float32)
    for c in range(NCORE):
        out[c * 512:(c + 1) * 512] = results[c]["out"]
    return out.reshape(B, N, DIM)


_NC_CACHE = []


def kernel(**inputs) -> np.ndarray:
    if not _NC_CACHE:
        _NC_CACHE.append(build())
    nc = _NC_CACHE[0]
    in_maps = prepare_in_maps(**inputs)
    res = run_bass_kernel_spmd(nc, in_maps, core_ids=list(range(NCORE)))
    return assemble_output(res.results)


# revision 43
# speedup vs baseline: 1.6102x; 1.0336x over previous
"""Trainium2 Bass kernel for nn_Attention_17008070493108.

Dense transformer attention block: QKV proj -> per-head LayerNorm -> RoPE
-> SDPA -> out proj, for x[2, 2048, 1024], H=16 heads, head_dim=64.

Sharding: tensor-parallel over heads. Each of the 8 NeuronCores owns 2
heads end-to-end (QKV column slices, norm, RoPE, attention). The
per-head context vectors are exchanged with a single AllToAll so each
core finishes the output projection (contraction over the full 1024
model dims) for its own 512-row slice of the output; the host
concatenates row slices.

Layouts (per core):
  xT          [1024, 4096] model-dim on partitions (host-transposed x)
  QT/KT       [128, 2048]x2 (batch-split) heads stacked on partitions
  scoresT     [128 keys, q] key tiles on partitions; softmax denominator
                          via a ones-column appended to V (ctx_aug row 64)
  ctx         [64, 4096]x2 -> AllToAll (bf16) -> out rows [512, 1024]

The emission is software-pipelined: LayerNorm+RoPE of batch 0 overlaps
the batch-1 projections on PE, and SDPA of batch 0 overlaps the batch-1
LayerNorm on DVE. Matmuls run in float32r (fp32 with 11-bit mantissa,
full PE rate); the output projection runs in bf16.
"""

import numpy as np

from concourse import bacc, tile, mybir
from concourse.bass_utils import run_bass_kernel_spmd

# ---------------------------------------------------------------- constants
DIM = 1024
H = 16
HD = 64
B = 2
N = 2048
R = B * N          # 4096 flattened rows
NCORE = 8
EPS = 1e-6

F32 = mybir.dt.float32
F32R = mybir.dt.float32r
BF16 = mybir.dt.bfloat16
ADD = mybir.AluOpType.add
SUB = mybir.AluOpType.subtract
MUL = mybir.AluOpType.mult

RC = R // 512        # 8 row chunks of 512
KT_DIM = DIM // 128  # 8 contraction tiles for the projections
NQC = N // 512       # 4 q chunks per batch
NKT = N // 128       # 16 key tiles per batch
VSTRIDE = 130        # per-keytile V_aug block: [vA(64) | 1 | vB(64) | 1]

DEBUG_OUTPUTS = False


def _round_fp32r(x: np.ndarray) -> np.ndarray:
    """Round fp32 to fp32r (11-bit mantissa, RNE)."""
    u = np.ascontiguousarray(x, dtype=np.float32).view(np.uint32)
    lsb = (u >> np.uint32(12)) & np.uint32(1)
    r = (u + np.uint32(0x7FF) + lsb) & np.uint32(0xFFFFF000)
    return r.view(np.float32)


# ---------------------------------------------------------------- graph
def build():
    nc = bacc.Bacc("TRN2", target_bir_lowering=False, debug=False,
                   num_devices=NCORE)

    # ---- DRAM parameters
    xT_d = nc.dram_tensor("xT", [DIM, R], F32R, kind="ExternalInput")
    wqkv_d = nc.dram_tensor("wqkv", [DIM, 3 * 128], F32R, kind="ExternalInput")
    bqkv_d = nc.dram_tensor("bqkv", [3, 128, 1], F32, kind="ExternalInput")
    # stats lhsT per chunk: [:, 0] x-sums cols {2jj+h}, [:, 1] sq-sums
    # cols {32+2jj+h}; both accumulate into one [40, 512] psum bank.
    onesblk_d = nc.dram_tensor("onesblk", [RC, 2, 128, 40], F32R,
                               kind="ExternalInput")
    wbln_d = nc.dram_tensor("wbln", [4, 128, 1], F32, kind="ExternalInput")
    cos_d = nc.dram_tensor("cosr", [128, R], F32, kind="ExternalInput")
    sinm_d = nc.dram_tensor("sinm", [128, R], F32, kind="ExternalInput")
    ident_d = nc.dram_tensor("ident", [128, 128], F32, kind="ExternalInput")
    ones_d = nc.dram_tensor("ones64", [128, 4 * NKT], F32R,
                            kind="ExternalInput")
    wo_d = nc.dram_tensor("wo", [DIM, DIM], BF16, kind="ExternalInput")
    borep_d = nc.dram_tensor("borep", [128, DIM], F32, kind="ExternalInput")
    out_d = nc.dram_tensor("out", [R // NCORE, DIM], F32, kind="ExternalOutput")
    if DEBUG_OUTPUTS:
        dbg_qrot = nc.dram_tensor("dbg_qrot", [128, R], F32,
                                  kind="ExternalOutput")
        dbg_krot = nc.dram_tensor("dbg_krot", [128, R], F32,
                                  kind="ExternalOutput")
        dbg_den = nc.dram_tensor("dbg_den", [16, 512], F32,
                                 kind="ExternalOutput")
        dbg_ctxn = nc.dram_tensor("dbg_ctxn", [128, R], F32,
                                  kind="ExternalOutput")

    with tile.TileContext(nc) as tc:
        with (
            tc.tile_pool(name="const", bufs=1) as cpool,
            tc.tile_pool(name="persist", bufs=1) as ppool,
            tc.tile_pool(name="chp", bufs=2) as chpool,
            tc.tile_pool(name="statp", bufs=8) as statpool,
            tc.tile_pool(name="stagp", bufs=4) as stagpool,
            tc.tile_pool(name="dram", bufs=1, space="DRAM") as dpool,
        ):
            # ---- constants in SBUF
            wqkv_sb = cpool.tile([128, KT_DIM, 384], F32R)
            nc.sync.dma_start(
                out=wqkv_sb[:],
                in_=wqkv_d.ap().rearrange("(k p) c -> p k c", p=128))
            bq_sb = cpool.tile([128, 1], F32)
            bk_sb = cpool.tile([128, 1], F32)
            bv_sb = cpool.tile([128, 1], F32)
            nc.sync.dma_start(out=bq_sb[:], in_=bqkv_d.ap()[0])
            nc.sync.dma_start(out=bk_sb[:], in_=bqkv_d.ap()[1])
            nc.sync.dma_start(out=bv_sb[:], in_=bqkv_d.ap()[2])
            onesblk_sb = cpool.tile([128, RC, 2, 40], F32R)
            nc.sync.dma_start(
                out=onesblk_sb[:],
                in_=onesblk_d.ap().rearrange("j s p c -> p j s c"))
            wlnq_sb = cpool.tile([128, 1], F32)
            blnq_sb = cpool.tile([128, 1], F32)
            wlnk_sb = cpool.tile([128, 1], F32)
            blnk_sb = cpool.tile([128, 1], F32)
            nc.sync.dma_start(out=wlnq_sb[:], in_=wbln_d.ap()[0])
            nc.sync.dma_start(out=blnq_sb[:], in_=wbln_d.ap()[1])
            nc.sync.dma_start(out=wlnk_sb[:], in_=wbln_d.ap()[2])
            nc.sync.dma_start(out=blnk_sb[:], in_=wbln_d.ap()[3])
            ident_sb = cpool.tile([128, 128], F32)
            nc.sync.dma_start(out=ident_sb[:], in_=ident_d.ap()[:, :])
            borep_sb = cpool.tile([128, DIM], F32)
            nc.sync.dma_start(out=borep_sb[:], in_=borep_d.ap()[:, :])

            # ---- persistent tensors (batch-split Q/K; in-place LN+RoPE)
            qkt = {}
            for g in range(B):
                qkt[("q", g)] = ppool.tile([128, N], F32R, tag=f"q{g}",
                                           name=f"qraw{g}")
                qkt[("k", g)] = ppool.tile([128, N], F32R, tag=f"k{g}",
                                           name=f"kraw{g}")
            vaug = ppool.tile([128, 2 * NKT * VSTRIDE], F32R, tag="vaug")
            ctxn_a = ppool.tile([64, R], BF16, tag="ctxn_a")
            ctxn_b = ppool.tile([64, R], BF16, tag="ctxn_b")

            nc.gpsimd.dma_start(
                out=vaug[:].rearrange("p (k c) -> p k c", c=65)[:, :, 64:65],
                in_=ones_d.ap()[:, :])

            stat_dr = {}
            ctxus = {}
            den_dr = dpool.tile([16, 512], BF16, name="den_dr")
            rec_dr = dpool.tile([16, 512], F32, name="rec_dr")
            denpacks = []

            # ---------------- emission helpers ----------------
            def emit_proj_row(r, xtpool, vchpool, ps1, ps1v, statps):
                """Project row-chunk r for q, k, v (+ inline stats MMs)."""
                g, jj = r // 4, r % 4
                xts = []
                for kt in range(KT_DIM):
                    xt = xtpool.tile([128, 512], F32R, tag="xt",
                                     name=f"xt_{r}_{kt}")
                    nc.sync.dma_start(
                        out=xt[:],
                        in_=xT_d.ap()[kt * 128:(kt + 1) * 128,
                                      r * 512:(r + 1) * 512])
                    xts.append(xt)
                for m, name, bias in ((0, "q", bq_sb), (1, "k", bk_sb),
                                      (2, "v", bv_sb)):
                    ps = ps1.tile([128, 512], F32, tag="proj",
                                  name=f"proj_{m}_{r}")
                    for kt in range(KT_DIM):
                        nc.tensor.matmul(
                            ps[:], wqkv_sb[:, kt, m * 128:(m + 1) * 128],
                            xts[kt][:],
                            start=(kt == 0), stop=(kt == KT_DIM - 1))
                    if m < 2:
                        dest = qkt[(name, g)]
                        nc.vector.tensor_scalar(
                            dest[:, jj * 512:(jj + 1) * 512], ps[:],
                            bias[:], None, ADD)
                        sps = statps[(name, g)]
                        nc.tensor.matmul(
                            sps[:], onesblk_sb[:, r, 0, :],
                            dest[:, jj * 512:(jj + 1) * 512],
                            start=(jj == 0), stop=False)
                        sqc = chpool.tile([128, 512], F32R, tag="sqc",
                                          name=f"sqc_{name}_{r}")
                        nc.vector.tensor_tensor(
                            sqc[:],
                            dest[:, jj * 512:(jj + 1) * 512].bitcast(F32),
                            dest[:, jj * 512:(jj + 1) * 512].bitcast(F32),
                            MUL)
                        nc.tensor.matmul(
                            sps[:], onesblk_sb[:, r, 1, :], sqc[:],
                            start=False, stop=(jj == 3))
                    else:
                        vch = vchpool.tile([128, 512], F32, tag="vch",
                                           name=f"vch_{r}")
                        nc.vector.tensor_scalar(vch[:], ps[:], bias[:],
                                                None, ADD)
                        for sseg in range(4):
                            kt_glob = r * 4 + sseg
                            tps = ps1v.tile([128, 128], F32, tag="vtr",
                                            name=f"vtr_{kt_glob}")
                            nc.tensor.transpose(
                                tps[:], vch[:, sseg * 128:(sseg + 1) * 128],
                                ident_sb[:])
                            vb = kt_glob * VSTRIDE
                            nc.vector.tensor_copy(
                                vaug[:, vb:vb + 64], tps[:, 0:64])
                            nc.vector.tensor_copy(
                                vaug[:, vb + 65:vb + 129], tps[:, 64:128])

            def emit_statmath(name, g, statps):
                """stat bank [40, 512]: rows 0-7 x-sums, 32-39 sq-sums ->
                rstd/murstd [8, 512] staged to DRAM."""
                sps = statps[(name, g)]
                mu = statpool.tile([8, 512], F32, tag="stat_sb",
                                   name=f"mu_{name}{g}")
                msqe = statpool.tile([8, 512], F32, tag="stat_sb",
                                     name=f"msqe_{name}{g}")
                nc.vector.tensor_scalar(mu[:], sps[0:8, :], 1.0 / HD,
                                        None, MUL)
                nc.vector.tensor_scalar(msqe[:], sps[32:40, :], 1.0 / HD,
                                        EPS, MUL, ADD)
                var = statpool.tile([8, 512], F32, tag="stat_sb",
                                    name=f"var_{name}{g}")
                nc.vector.tensor_tensor(var[:], mu[:], mu[:], MUL)
                nc.vector.tensor_tensor(var[:], msqe[:], var[:], SUB)
                sd = statpool.tile([8, 512], F32, tag="stat_sb",
                                   name=f"sd_{name}{g}")
                nc.scalar.activation(sd[:], var[:],
                                     mybir.ActivationFunctionType.Sqrt)
                rstd = statpool.tile([8, 512], F32, tag="stat_sb",
                                     name=f"rstd_{name}{g}")
                nc.vector.reciprocal(rstd[:], sd[:])
                murstd = statpool.tile([8, 512], F32, tag="stat_sb",
                                       name=f"murstd_{name}{g}")
                nc.vector.tensor_tensor(murstd[:], mu[:], rstd[:], MUL)
                rdr = dpool.tile([8, 512], F32, name=f"rstd_dr_{name}{g}")
                mdr = dpool.tile([8, 512], F32, name=f"mur_dr_{name}{g}")
                nc.sync.dma_start(out=rdr[:], in_=rstd[:])
                nc.sync.dma_start(out=mdr[:], in_=murstd[:])
                stat_dr[(name, g)] = (rdr, mdr)

            def emit_apply(name, g, jj, w_sb, b_sb):
                """LN apply + RoPE for chunk jj of batch g (in place)."""
                traw = qkt[(name, g)]
                rdr, mdr = stat_dr[(name, g)]
                jsl = slice(jj * 512, (jj + 1) * 512)
                gsl = slice(g * N + jj * 512, g * N + (jj + 1) * 512)
                cosc = chpool.tile([128, 512], F32, tag="cosc",
                                   name=f"cosc_{name}_{g}{jj}")
                sinc = chpool.tile([128, 512], F32, tag="sinc",
                                   name=f"sinc_{name}_{g}{jj}")
                nc.sync.dma_start(out=cosc[:], in_=cos_d.ap()[:, gsl])
                nc.sync.dma_start(out=sinc[:], in_=sinm_d.ap()[:, gsl])
                rep_r = chpool.tile([128, 512], F32, tag="rep_r",
                                    name=f"rep_r_{name}_{g}{jj}")
                rep_m = chpool.tile([128, 512], F32, tag="rep_m",
                                    name=f"rep_m_{name}_{g}{jj}")
                for h in range(2):
                    stg_r = stagpool.tile([1, 512], F32, tag="stg",
                                          name=f"sr_{name}_{g}{jj}_{h}")
                    stg_m = stagpool.tile([1, 512], F32, tag="stg",
                                          name=f"sm_{name}_{g}{jj}_{h}")
                    nc.sync.dma_start(out=stg_r[:], in_=rdr[2 * jj + h])
                    nc.sync.dma_start(out=stg_m[:], in_=mdr[2 * jj + h])
                    if h == 0:
                        nc.gpsimd.partition_broadcast(
                            rep_r[0:64, :], stg_r[:], channels=64)
                        nc.gpsimd.partition_broadcast(
                            rep_m[0:64, :], stg_m[:], channels=64)
                    else:
                        tmp_r = stagpool.tile([64, 512], F32, tag="tmpb",
                                              name=f"tr_{name}_{g}{jj}")
                        tmp_m = stagpool.tile([64, 512], F32, tag="tmpb",
                                              name=f"tm_{name}_{g}{jj}")
                        nc.gpsimd.partition_broadcast(
                            tmp_r[:], stg_r[:], channels=64)
                        nc.gpsimd.partition_broadcast(
                            tmp_m[:], stg_m[:], channels=64)
                        nc.sync.dma_start(out=rep_r[64:128, :],
                                          in_=tmp_r[:])
                        nc.sync.dma_start(out=rep_m[64:128, :],
                                          in_=tmp_m[:])
                tn = chpool.tile([128, 512], F32, tag="tn",
                                 name=f"tn_{name}_{g}{jj}")
                nc.vector.tensor_tensor(tn[:], traw[:, jsl].bitcast(F32),
                                        rep_r[:], MUL)
                nc.vector.tensor_tensor(tn[:], tn[:], rep_m[:], SUB)
                nc.vector.tensor_scalar(tn[:], tn[:], w_sb[:], b_sb[:],
                                        MUL, ADD)
                swp = chpool.tile([128, 512], F32, tag="swp",
                                  name=f"swp_{name}_{g}{jj}")
                for (dst, src) in ((0, 32), (32, 0), (64, 96), (96, 64)):
                    nc.sync.dma_start(out=swp[dst:dst + 32, :],
                                      in_=tn[src:src + 32, :])
                t1 = chpool.tile([128, 512], F32, tag="t1",
                                 name=f"t1_{name}_{g}{jj}")
                nc.vector.tensor_tensor(t1[:], tn[:], cosc[:], MUL)
                nc.vector.tensor_tensor(swp[:], swp[:], sinc[:], MUL)
                nc.vector.tensor_tensor(traw[:, jsl], t1[:], swp[:], ADD)

            def emit_sdpa_qcpair(g, qc0, exppool, ctxupool, sp3,
                                 ps_sc, ps_ctx):
                """SDPA for q-chunks (qc0, qc0+1): kt-outer so the K/V
                stationary tiles are reused across both q-chunks, then
                per-pair normalize with a reshaped (cheap) reciprocal."""
                qrot = qkt[("q", g)]
                krot = qkt[("k", g)]
                qcs = (qc0, qc0 + 1)
                ctxps = {}
                for qc in qcs:
                    ctxps[(qc, 0)] = ps_ctx.tile([65, 512], F32, tag="ctx",
                                                 name=f"ctxa_{g}{qc}")
                    ctxps[(qc, 1)] = ps_ctx.tile([65, 512], F32, tag="ctx",
                                                 name=f"ctxb_{g}{qc}")
                for kt in range(NKT):
                    ksl = slice(kt * 128, (kt + 1) * 128)
                    vbase = (g * NKT + kt) * VSTRIDE
                    scs = {}
                    for qc in qcs:
                        qsl = slice(qc * 512, (qc + 1) * 512)
                        scps = ps_sc.tile([128, 1024], F32, tag="sc",
                                          name=f"sc_{g}{qc}{kt}")
                        scs[qc] = scps
                    # QK: stationary K tile reused across both q-chunks
                    for h, psl in ((0, slice(0, 64)), (1, slice(64, 128))):
                        for qc in qcs:
                            qsl = slice(qc * 512, (qc + 1) * 512)
                            nc.tensor.matmul(
                                scs[qc][:, h * 512:(h + 1) * 512],
                                krot[psl, ksl], qrot[psl, qsl],
                                start=True, stop=True,
                                tile_position=(h * 64, 0))
                    for qc in qcs:
                        expt = exppool.tile([128, 1024], F32R, tag="expt",
                                            name=f"ex_{g}{qc}{kt}")
                        nc.scalar.activation(
                            expt[:], scs[qc][:],
                            mybir.ActivationFunctionType.Exp,
                            scale=float(HD) ** -0.5)
                        scs[qc] = expt
                    # PV: stationary V_aug tile reused across both q-chunks
                    for h in range(2):
                        vsl = slice(vbase + h * 65, vbase + (h + 1) * 65)
                        for qc in qcs:
                            nc.tensor.matmul(
                                ctxps[(qc, h)][:], vaug[:, vsl],
                                scs[qc][:, h * 512:(h + 1) * 512],
                                start=(kt == 0), stop=(kt == NKT - 1))
                for qc in qcs:
                    cua = ctxupool.tile([65, 512], BF16, tag="ctxu",
                                        name=f"cua_{g}{qc}")
                    cub = ctxupool.tile([65, 512], BF16, tag="ctxu",
                                        name=f"cub_{g}{qc}")
                    nc.vector.tensor_copy(cua[:], ctxps[(qc, 0)][:])
                    nc.vector.tensor_copy(cub[:], ctxps[(qc, 1)][:])
                    idx = g * 8 + qc * 2
                    nc.gpsimd.dma_start(out=den_dr[idx], in_=cua[64:65, :])
                    nc.gpsimd.dma_start(out=den_dr[idx + 1],
                                        in_=cub[64:65, :])
                    ctxus[(g, qc)] = (cua, cub)
                # per-pair normalize (denoms of both q-chunks at once)
                idx0 = g * 8 + qc0 * 2
                dp = sp3.tile([4, 512], BF16, tag="denpack",
                              name=f"dp{g}{qc0}")
                nc.sync.dma_start(out=dp[:],
                                  in_=den_dr.opt()[idx0:idx0 + 4, :])
                rc = sp3.tile([4, 512], F32, tag="recip",
                              name=f"rc{g}{qc0}")
                nc.vector.reciprocal(rc[:], dp[:])
                nc.sync.dma_start(out=rec_dr[idx0:idx0 + 4, :], in_=rc[:])
                for qc in qcs:
                    cua, cub = ctxus[(g, qc)]
                    qsl = slice(g * N + qc * 512, g * N + (qc + 1) * 512)
                    for h, cu, dst in ((0, cua, ctxn_a), (1, cub, ctxn_b)):
                        stg = sp3.tile([1, 512], F32, tag="stg3",
                                       name=f"st{g}{qc}{h}")
                        nc.sync.dma_start(
                            out=stg[:], in_=rec_dr[g * 8 + qc * 2 + h])
                        rep = sp3.tile([64, 512], F32, tag="rep",
                                       name=f"rp{g}{qc}{h}")
                        nc.gpsimd.partition_broadcast(rep[:], stg[:],
                                                      channels=64)
                        nc.vector.tensor_tensor(dst[:, qsl], cu[0:64, :],
                                                rep[:], MUL)

            # ---------------- pipelined emission ----------------
            with (
                tc.tile_pool(name="xtp", bufs=8) as xtpool,
                tc.tile_pool(name="vchp", bufs=3) as vchpool,
                tc.tile_pool(name="ps1", bufs=3, space="PSUM") as ps1,
                tc.tile_pool(name="ps1v", bufs=1, space="PSUM") as ps1v,
                tc.tile_pool(name="ps2", bufs=2, space="PSUM") as ps2,
            ):
                statps = {}
                for tname in ("q", "k"):
                    statps[(tname, 0)] = ps2.tile(
                        [40, 512], F32, tag="stat", name=f"stat_{tname}0")
                for r in range(4):
                    emit_proj_row(r, xtpool, vchpool, ps1, ps1v, statps)
                emit_statmath("q", 0, statps)
                emit_statmath("k", 0, statps)
                # batch-0 LN interleaved with batch-1 projections
                for tname in ("q", "k"):
                    statps[(tname, 1)] = ps2.tile(
                        [40, 512], F32, tag="stat", name=f"stat_{tname}1")
                for jj in range(4):
                    emit_proj_row(4 + jj, xtpool, vchpool, ps1, ps1v, statps)
                    emit_apply("q", 0, jj, wlnq_sb, blnq_sb)
                    emit_apply("k", 0, jj, wlnk_sb, blnk_sb)
                emit_statmath("q", 1, statps)
                emit_statmath("k", 1, statps)

            # batch-1 LN interleaved with batch-0 SDPA
            with (
                tc.tile_pool(name="expp", bufs=3) as exppool,
                tc.tile_pool(name="ctxup", bufs=4) as ctxupool,
                tc.tile_pool(name="sp3", bufs=2) as sp3,
                tc.tile_pool(name="ps_sc", bufs=2, space="PSUM") as ps_sc,
                tc.tile_pool(name="ps_ctx", bufs=4, space="PSUM") as ps_ctx,
            ):
                emit_apply("q", 1, 0, wlnq_sb, blnq_sb)
                emit_apply("k", 1, 0, wlnk_sb, blnk_sb)
                emit_apply("q", 1, 1, wlnq_sb, blnq_sb)
                emit_sdpa_qcpair(0, 0, exppool, ctxupool, sp3, ps_sc,
                                 ps_ctx)
                emit_apply("k", 1, 1, wlnk_sb, blnk_sb)
                emit_apply("q", 1, 2, wlnq_sb, blnq_sb)
                emit_apply("k", 1, 2, wlnk_sb, blnk_sb)
                emit_sdpa_qcpair(0, 2, exppool, ctxupool, sp3, ps_sc,
                                 ps_ctx)
                emit_apply("q", 1, 3, wlnq_sb, blnq_sb)
                emit_apply("k", 1, 3, wlnk_sb, blnk_sb)
                emit_sdpa_qcpair(1, 0, exppool, ctxupool, sp3, ps_sc,
                                 ps_ctx)
                emit_sdpa_qcpair(1, 2, exppool, ctxupool, sp3, ps_sc,
                                 ps_ctx)

            # ================= AllToAll (bf16) =================
            a2a_in = dpool.tile([NCORE, 128, 512], BF16)
            a2a_out = dpool.tile([NCORE, 128, 512], BF16)
            for j in range(NCORE):
                nc.gpsimd.dma_start(out=a2a_in[j][0:64, :],
                                    in_=ctxn_a[:, j * 512:(j + 1) * 512])
                nc.gpsimd.dma_start(out=a2a_in[j][64:128, :],
                                    in_=ctxn_b[:, j * 512:(j + 1) * 512])
            nc.gpsimd.collective_compute(
                "AllToAll", mybir.AluOpType.bypass,
                ins=[a2a_in.opt()], outs=[a2a_out.opt()],
                replica_groups=[list(range(NCORE))],
            )

            # ================= output projection (bf16) ==============
            with (
                tc.tile_pool(name="wop", bufs=3) as wopool,
                tc.tile_pool(name="sp5", bufs=4) as sp5,
                tc.tile_pool(name="ps_out", bufs=4, space="PSUM") as ps_out,
            ):
                ops = [ps_out.tile([128, 1024], F32, tag="outp",
                                   name=f"outp{i}") for i in range(4)]
                for kt in range(KT_DIM):
                    wo_sb = wopool.tile([128, DIM], BF16, tag="wo",
                                        name=f"wo{kt}")
                    nc.sync.dma_start(
                        out=wo_sb[:],
                        in_=wo_d.ap()[kt * 128:(kt + 1) * 128, :])
                    cg = wopool.tile([128, 512], BF16, tag="ctxg",
                                     name=f"cg{kt}")
                    nc.sync.dma_start(out=cg[:], in_=a2a_out[kt])
                    for rt in range(4):
                        for nh in range(2):
                            nc.tensor.matmul(
                                ops[rt][:, nh * 512:(nh + 1) * 512],
                                cg[:, rt * 128:(rt + 1) * 128],
                                wo_sb[:, nh * 512:(nh + 1) * 512],
                                start=(kt == 0), stop=(kt == KT_DIM - 1))
                for rt in range(4):
                    osb = sp5.tile([128, DIM], F32, tag="osb",
                                   name=f"osb{rt}")
                    nc.vector.tensor_tensor(osb[:], ops[rt][:], borep_sb[:],
                                            ADD)
                    nc.sync.dma_start(
                        out=out_d.ap()[rt * 128:(rt + 1) * 128, :],
                        in_=osb[:])

            if DEBUG_OUTPUTS:
                for g in range(B):
                    nc.sync.dma_start(
                        out=dbg_qrot.ap()[:, g * N:(g + 1) * N],
                        in_=qkt[("q", g)][:].bitcast(F32))
                    nc.sync.dma_start(
                        out=dbg_krot.ap()[:, g * N:(g + 1) * N],
                        in_=qkt[("k", g)][:].bitcast(F32))

            if DEBUG_OUTPUTS:
                nc.gpsimd.dma_start(out=dbg_den.ap()[:, :], in_=den_dr.opt())
                nc.gpsimd.dma_start(out=dbg_ctxn.ap()[0:64, :], in_=ctxn_a[:])
                nc.gpsimd.dma_start(out=dbg_ctxn.ap()[64:128, :],
                                    in_=ctxn_b[:])

    nc.compile()
    return nc


# ---------------------------------------------------------------- host side
def prepare_in_maps(x, rotary_cos, rotary_sin, Wq, bq, Wk, bk, Wv, bv,
                    q_norm_w, q_norm_b, k_norm_w, k_norm_b, Wo, bo):
    import ml_dtypes

    x = np.asarray(x, np.float32)
    xT = _round_fp32r(np.ascontiguousarray(x.reshape(R, DIM).T))

    Wcat = np.concatenate([np.asarray(Wq, np.float32),
                           np.asarray(Wk, np.float32),
                           np.asarray(Wv, np.float32)], axis=1)
    bcat = np.concatenate([np.asarray(bq, np.float32),
                           np.asarray(bk, np.float32),
                           np.asarray(bv, np.float32)])

    def head_cols(h, part):
        s = 192 * h + 64 * part
        return np.arange(s, s + 64)

    cos_flat = np.asarray(rotary_cos, np.float32).reshape(R, HD).T
    sin_flat = np.asarray(rotary_sin, np.float32).reshape(R, HD).T
    sinm = sin_flat.copy()
    sinm[0:32] = -sin_flat[0:32]
    cos_rep = np.ascontiguousarray(np.tile(cos_flat, (2, 1)))
    sinm_rep = np.ascontiguousarray(np.tile(sinm, (2, 1)))

    onesblk = np.zeros((RC, 2, 128, 40), np.float32)
    for j in range(RC):
        jj = j % 4
        onesblk[j, 0, 0:64, 2 * jj] = 1.0
        onesblk[j, 0, 64:128, 2 * jj + 1] = 1.0
        onesblk[j, 1, 0:64, 32 + 2 * jj] = 1.0
        onesblk[j, 1, 64:128, 32 + 2 * jj + 1] = 1.0

    wbln = np.stack([
        np.tile(np.asarray(q_norm_w, np.float32), 2)[:, None],
        np.tile(np.asarray(q_norm_b, np.float32), 2)[:, None],
        np.tile(np.asarray(k_norm_w, np.float32), 2)[:, None],
        np.tile(np.asarray(k_norm_b, np.float32), 2)[:, None],
    ])

    ident = np.eye(128, dtype=np.float32)
    ones64 = np.ones((128, 4 * NKT), np.float32)
    borep = np.tile(np.asarray(bo, np.float32)[None, :], (128, 1))
    wo_bf = np.asarray(Wo, np.float32).astype(ml_dtypes.bfloat16)

    in_maps = []
    for c in range(NCORE):
        hA, hB = 2 * c, 2 * c + 1
        cols = np.concatenate([
            head_cols(hA, 0), head_cols(hB, 0),
            head_cols(hA, 1), head_cols(hB, 1),
            head_cols(hA, 2), head_cols(hB, 2),
        ])
        wqkv_c = _round_fp32r(np.ascontiguousarray(Wcat[:, cols]))
        bqkv_c = np.ascontiguousarray(bcat[cols].reshape(3, 128, 1))
        in_maps.append({
            "xT": xT,
            "wqkv": wqkv_c,
            "bqkv": bqkv_c,
            "onesblk": onesblk,
            "wbln": wbln,
            "cosr": cos_rep,
            "sinm": sinm_rep,
            "ident": ident,
            "ones64": ones64,
            "wo": wo_bf,
            "borep": borep,
        })
    return in_maps


def assemble_output(results):
    out = np.empty((R, DIM), np.float32)
    for c in range(NCORE):
        out[c * 512:(c + 1) * 512] = results[c]["out"]
    return out.reshape(B, N, DIM)


_NC_CACHE = []


def kernel(**inputs) -> np.ndarray:
    if not _NC_CACHE:
        _NC_CACHE.append(build())
    nc = _NC_CACHE[0]
    in_maps = prepare_in_maps(**inputs)
    res = run_bass_kernel_spmd(nc, in_maps, core_ids=list(range(NCORE)))
    return assemble_output(res.results)


# revision 44
# speedup vs baseline: 1.6285x; 1.0114x over previous
"""Trainium2 Bass kernel for nn_Attention_17008070493108.

Dense transformer attention block: QKV proj -> per-head LayerNorm -> RoPE
-> SDPA -> out proj, for x[2, 2048, 1024], H=16 heads, head_dim=64.

Sharding: tensor-parallel over heads. Each of the 8 NeuronCores owns 2
heads end-to-end (QKV column slices, norm, RoPE, attention). The
per-head context vectors are exchanged with a single AllToAll so each
core finishes the output projection (contraction over the full 1024
model dims) for its own 512-row slice of the output; the host
concatenates row slices.

Layouts (per core):
  xT          [1024, 4096] model-dim on partitions (host-transposed x)
  QT/KT       [128, 2048]x2 (batch-split) heads stacked on partitions
  scoresT     [128 keys, q] key tiles on partitions; softmax denominator
                          via a ones-column appended to V (ctx_aug row 64)
  ctx         [64, 4096]x2 -> AllToAll (bf16) -> out rows [512, 1024]

The emission is software-pipelined: LayerNorm+RoPE of batch 0 overlaps
the batch-1 projections on PE, and SDPA of batch 0 overlaps the batch-1
LayerNorm on DVE. Matmuls run in float32r (fp32 with 11-bit mantissa,
full PE rate); the output projection runs in bf16.
"""

import numpy as np

from concourse import bacc, tile, mybir
from concourse.bass_utils import run_bass_kernel_spmd

# ---------------------------------------------------------------- constants
DIM = 1024
H = 16
HD = 64
B = 2
N = 2048
R = B * N          # 4096 flattened rows
NCORE = 8
EPS = 1e-6

F32 = mybir.dt.float32
F32R = mybir.dt.float32r
BF16 = mybir.dt.bfloat16
ADD = mybir.AluOpType.add
SUB = mybir.AluOpType.subtract
MUL = mybir.AluOpType.mult

RC = R // 512        # 8 row chunks of 512
KT_DIM = DIM // 128  # 8 contraction tiles for the projections
NQC = N // 512       # 4 q chunks per batch
NKT = N // 128       # 16 key tiles per batch
VSTRIDE = 130        # per-keytile V_aug block: [vA(64) | 1 | vB(64) | 1]

DEBUG_OUTPUTS = False


def _round_fp32r(x: np.ndarray) -> np.ndarray:
    """Round fp32 to fp32r (11-bit mantissa, RNE)."""
    u = np.ascontiguousarray(x, dtype=np.float32).view(np.uint32)
    lsb = (u >> np.uint32(12)) & np.uint32(1)
    r = (u + np.uint32(0x7FF) + lsb) & np.uint32(0xFFFFF000)
    return r.view(np.float32)


# ---------------------------------------------------------------- graph
def build():
    nc = bacc.Bacc("TRN2", target_bir_lowering=False, debug=False,
                   num_devices=NCORE)

    # ---- DRAM parameters
    xT_d = nc.dram_tensor("xT", [DIM, R], F32R, kind="ExternalInput")
    wqkv_d = nc.dram_tensor("wqkv", [DIM, 3 * 128], F32R, kind="ExternalInput")
    bqkv_d = nc.dram_tensor("bqkv", [3, 128, 1], F32, kind="ExternalInput")
    # stats lhsT per chunk: [:, 0] x-sums cols {2jj+h}, [:, 1] sq-sums
    # cols {32+2jj+h}; both accumulate into one [40, 512] psum bank.
    onesblk_d = nc.dram_tensor("onesblk", [RC, 2, 128, 40], F32R,
                               kind="ExternalInput")
    wbln_d = nc.dram_tensor("wbln", [4, 128, 1], F32, kind="ExternalInput")
    cos_d = nc.dram_tensor("cosr", [128, R], F32, kind="ExternalInput")
    sinm_d = nc.dram_tensor("sinm", [128, R], F32, kind="ExternalInput")
    ident_d = nc.dram_tensor("ident", [128, 128], F32, kind="ExternalInput")
    ones_d = nc.dram_tensor("ones64", [128, 4 * NKT], F32R,
                            kind="ExternalInput")
    wo_d = nc.dram_tensor("wo", [DIM, DIM], BF16, kind="ExternalInput")
    borep_d = nc.dram_tensor("borep", [128, DIM], F32, kind="ExternalInput")
    out_d = nc.dram_tensor("out", [R // NCORE, DIM], F32, kind="ExternalOutput")
    if DEBUG_OUTPUTS:
        dbg_qrot = nc.dram_tensor("dbg_qrot", [128, R], F32,
                                  kind="ExternalOutput")
        dbg_krot = nc.dram_tensor("dbg_krot", [128, R], F32,
                                  kind="ExternalOutput")
        dbg_den = nc.dram_tensor("dbg_den", [16, 512], F32,
                                 kind="ExternalOutput")
        dbg_ctxn = nc.dram_tensor("dbg_ctxn", [128, R], F32,
                                  kind="ExternalOutput")

    with tile.TileContext(nc) as tc:
        with (
            tc.tile_pool(name="const", bufs=1) as cpool,
            tc.tile_pool(name="persist", bufs=1) as ppool,
            tc.tile_pool(name="chp", bufs=2) as chpool,
            tc.tile_pool(name="statp", bufs=8) as statpool,
            tc.tile_pool(name="stagp", bufs=4) as stagpool,
            tc.tile_pool(name="dram", bufs=1, space="DRAM") as dpool,
        ):
            # ---- constants in SBUF
            wqkv_sb = cpool.tile([128, KT_DIM, 384], F32R)
            nc.sync.dma_start(
                out=wqkv_sb[:],
                in_=wqkv_d.ap().rearrange("(k p) c -> p k c", p=128))
            bq_sb = cpool.tile([128, 1], F32)
            bk_sb = cpool.tile([128, 1], F32)
            bv_sb = cpool.tile([128, 1], F32)
            nc.sync.dma_start(out=bq_sb[:], in_=bqkv_d.ap()[0])
            nc.sync.dma_start(out=bk_sb[:], in_=bqkv_d.ap()[1])
            nc.sync.dma_start(out=bv_sb[:], in_=bqkv_d.ap()[2])
            onesblk_sb = cpool.tile([128, RC, 2, 40], F32R)
            nc.sync.dma_start(
                out=onesblk_sb[:],
                in_=onesblk_d.ap().rearrange("j s p c -> p j s c"))
            wlnq_sb = cpool.tile([128, 1], F32)
            blnq_sb = cpool.tile([128, 1], F32)
            wlnk_sb = cpool.tile([128, 1], F32)
            blnk_sb = cpool.tile([128, 1], F32)
            nc.sync.dma_start(out=wlnq_sb[:], in_=wbln_d.ap()[0])
            nc.sync.dma_start(out=blnq_sb[:], in_=wbln_d.ap()[1])
            nc.sync.dma_start(out=wlnk_sb[:], in_=wbln_d.ap()[2])
            nc.sync.dma_start(out=blnk_sb[:], in_=wbln_d.ap()[3])
            ident_sb = cpool.tile([128, 128], F32)
            nc.sync.dma_start(out=ident_sb[:], in_=ident_d.ap()[:, :])
            borep_sb = cpool.tile([128, DIM], F32)
            nc.sync.dma_start(out=borep_sb[:], in_=borep_d.ap()[:, :])

            # ---- persistent tensors (batch-split Q/K; in-place LN+RoPE)
            qkt = {}
            for g in range(B):
                qkt[("q", g)] = ppool.tile([128, N], F32R, tag=f"q{g}",
                                           name=f"qraw{g}")
                qkt[("k", g)] = ppool.tile([128, N], F32R, tag=f"k{g}",
                                           name=f"kraw{g}")
            vaug = ppool.tile([128, 2 * NKT * VSTRIDE], F32R, tag="vaug")
            ctxn_a = ppool.tile([64, R], BF16, tag="ctxn_a")
            ctxn_b = ppool.tile([64, R], BF16, tag="ctxn_b")

            nc.gpsimd.dma_start(
                out=vaug[:].rearrange("p (k c) -> p k c", c=65)[:, :, 64:65],
                in_=ones_d.ap()[:, :])

            stat_dr = {}
            ctxus = {}
            den_dr = dpool.tile([16, 512], BF16, name="den_dr")
            rec_dr = dpool.tile([16, 512], F32, name="rec_dr")
            denpacks = []

            # ---------------- emission helpers ----------------
            def emit_proj_row(r, xtpool, vchpool, ps1, ps1v, statps):
                """Project row-chunk r for q, k, v (+ inline stats MMs)."""
                g, jj = r // 4, r % 4
                xts = []
                for kt in range(KT_DIM):
                    xt = xtpool.tile([128, 512], F32R, tag="xt",
                                     name=f"xt_{r}_{kt}")
                    nc.sync.dma_start(
                        out=xt[:],
                        in_=xT_d.ap()[kt * 128:(kt + 1) * 128,
                                      r * 512:(r + 1) * 512])
                    xts.append(xt)
                for m, name, bias in ((0, "q", bq_sb), (1, "k", bk_sb),
                                      (2, "v", bv_sb)):
                    ps = ps1.tile([128, 512], F32, tag="proj",
                                  name=f"proj_{m}_{r}")
                    for kt in range(KT_DIM):
                        nc.tensor.matmul(
                            ps[:], wqkv_sb[:, kt, m * 128:(m + 1) * 128],
                            xts[kt][:],
                            start=(kt == 0), stop=(kt == KT_DIM - 1))
                    if m < 2:
                        dest = qkt[(name, g)]
                        nc.vector.tensor_scalar(
                            dest[:, jj * 512:(jj + 1) * 512], ps[:],
                            bias[:], None, ADD)
                        sps = statps[(name, g)]
                        nc.tensor.matmul(
                            sps[:], onesblk_sb[:, r, 0, :],
                            dest[:, jj * 512:(jj + 1) * 512],
                            start=(jj == 0), stop=False)
                        sqc = chpool.tile([128, 512], F32R, tag="sqc",
                                          name=f"sqc_{name}_{r}")
                        nc.scalar.square(
                            sqc[:],
                            dest[:, jj * 512:(jj + 1) * 512].bitcast(F32))
                        nc.tensor.matmul(
                            sps[:], onesblk_sb[:, r, 1, :], sqc[:],
                            start=False, stop=(jj == 3))
                    else:
                        vch = vchpool.tile([128, 512], F32, tag="vch",
                                           name=f"vch_{r}")
                        nc.scalar.add(vch[:], ps[:], bias[:])
                        for sseg in range(4):
                            kt_glob = r * 4 + sseg
                            tps = ps1v.tile([128, 128], F32, tag="vtr",
                                            name=f"vtr_{kt_glob}")
                            nc.tensor.transpose(
                                tps[:], vch[:, sseg * 128:(sseg + 1) * 128],
                                ident_sb[:])
                            vb = kt_glob * VSTRIDE
                            nc.scalar.copy(
                                vaug[:, vb:vb + 64], tps[:, 0:64])
                            nc.scalar.copy(
                                vaug[:, vb + 65:vb + 129], tps[:, 64:128])

            def emit_statmath(name, g, statps):
                """stat bank [40, 512]: rows 0-7 x-sums, 32-39 sq-sums ->
                rstd/murstd [8, 512] staged to DRAM."""
                sps = statps[(name, g)]
                mu = statpool.tile([8, 512], F32, tag="stat_sb",
                                   name=f"mu_{name}{g}")
                msqe = statpool.tile([8, 512], F32, tag="stat_sb",
                                     name=f"msqe_{name}{g}")
                nc.vector.tensor_scalar(mu[:], sps[0:8, :], 1.0 / HD,
                                        None, MUL)
                nc.vector.tensor_scalar(msqe[:], sps[32:40, :], 1.0 / HD,
                                        EPS, MUL, ADD)
                var = statpool.tile([8, 512], F32, tag="stat_sb",
                                    name=f"var_{name}{g}")
                nc.vector.tensor_tensor(var[:], mu[:], mu[:], MUL)
                nc.vector.tensor_tensor(var[:], msqe[:], var[:], SUB)
                sd = statpool.tile([8, 512], F32, tag="stat_sb",
                                   name=f"sd_{name}{g}")
                nc.scalar.activation(sd[:], var[:],
                                     mybir.ActivationFunctionType.Sqrt)
                rstd = statpool.tile([8, 512], F32, tag="stat_sb",
                                     name=f"rstd_{name}{g}")
                nc.vector.reciprocal(rstd[:], sd[:])
                murstd = statpool.tile([8, 512], F32, tag="stat_sb",
                                       name=f"murstd_{name}{g}")
                nc.vector.tensor_tensor(murstd[:], mu[:], rstd[:], MUL)
                rdr = dpool.tile([8, 512], F32, name=f"rstd_dr_{name}{g}")
                mdr = dpool.tile([8, 512], F32, name=f"mur_dr_{name}{g}")
                nc.sync.dma_start(out=rdr[:], in_=rstd[:])
                nc.sync.dma_start(out=mdr[:], in_=murstd[:])
                stat_dr[(name, g)] = (rdr, mdr)

            def emit_apply(name, g, jj, w_sb, b_sb):
                """LN apply + RoPE for chunk jj of batch g (in place)."""
                traw = qkt[(name, g)]
                rdr, mdr = stat_dr[(name, g)]
                jsl = slice(jj * 512, (jj + 1) * 512)
                gsl = slice(g * N + jj * 512, g * N + (jj + 1) * 512)
                cosc = chpool.tile([128, 512], F32, tag="cosc",
                                   name=f"cosc_{name}_{g}{jj}")
                sinc = chpool.tile([128, 512], F32, tag="sinc",
                                   name=f"sinc_{name}_{g}{jj}")
                nc.sync.dma_start(out=cosc[:], in_=cos_d.ap()[:, gsl])
                nc.sync.dma_start(out=sinc[:], in_=sinm_d.ap()[:, gsl])
                rep_r = chpool.tile([128, 512], F32, tag="rep_r",
                                    name=f"rep_r_{name}_{g}{jj}")
                rep_m = chpool.tile([128, 512], F32, tag="rep_m",
                                    name=f"rep_m_{name}_{g}{jj}")
                for h in range(2):
                    stg_r = stagpool.tile([1, 512], F32, tag="stg",
                                          name=f"sr_{name}_{g}{jj}_{h}")
                    stg_m = stagpool.tile([1, 512], F32, tag="stg",
                                          name=f"sm_{name}_{g}{jj}_{h}")
                    nc.sync.dma_start(out=stg_r[:], in_=rdr[2 * jj + h])
                    nc.sync.dma_start(out=stg_m[:], in_=mdr[2 * jj + h])
                    if h == 0:
                        nc.gpsimd.partition_broadcast(
                            rep_r[0:64, :], stg_r[:], channels=64)
                        nc.gpsimd.partition_broadcast(
                            rep_m[0:64, :], stg_m[:], channels=64)
                    else:
                        tmp_r = stagpool.tile([64, 512], F32, tag="tmpb",
                                              name=f"tr_{name}_{g}{jj}")
                        tmp_m = stagpool.tile([64, 512], F32, tag="tmpb",
                                              name=f"tm_{name}_{g}{jj}")
                        nc.gpsimd.partition_broadcast(
                            tmp_r[:], stg_r[:], channels=64)
                        nc.gpsimd.partition_broadcast(
                            tmp_m[:], stg_m[:], channels=64)
                        nc.sync.dma_start(out=rep_r[64:128, :],
                                          in_=tmp_r[:])
                        nc.sync.dma_start(out=rep_m[64:128, :],
                                          in_=tmp_m[:])
                tn = chpool.tile([128, 512], F32, tag="tn",
                                 name=f"tn_{name}_{g}{jj}")
                nc.vector.tensor_tensor(tn[:], traw[:, jsl].bitcast(F32),
                                        rep_r[:], MUL)
                nc.vector.tensor_tensor(tn[:], tn[:], rep_m[:], SUB)
                nc.vector.tensor_scalar(tn[:], tn[:], w_sb[:], b_sb[:],
                                        MUL, ADD)
                swp = chpool.tile([128, 512], F32, tag="swp",
                                  name=f"swp_{name}_{g}{jj}")
                for (dst, src) in ((0, 32), (32, 0), (64, 96), (96, 64)):
                    nc.sync.dma_start(out=swp[dst:dst + 32, :],
                                      in_=tn[src:src + 32, :])
                t1 = chpool.tile([128, 512], F32, tag="t1",
                                 name=f"t1_{name}_{g}{jj}")
                nc.vector.tensor_tensor(t1[:], tn[:], cosc[:], MUL)
                nc.vector.tensor_tensor(swp[:], swp[:], sinc[:], MUL)
                nc.vector.tensor_tensor(traw[:, jsl], t1[:], swp[:], ADD)

            def emit_sdpa_qcpair(g, qc0, exppool, ctxupool, sp3,
                                 ps_sc, ps_ctx):
                """SDPA for q-chunks (qc0, qc0+1): kt-outer so the K/V
                stationary tiles are reused across both q-chunks, then
                per-pair normalize with a reshaped (cheap) reciprocal."""
                qrot = qkt[("q", g)]
                krot = qkt[("k", g)]
                qcs = (qc0, qc0 + 1)
                ctxps = {}
                for qc in qcs:
                    ctxps[(qc, 0)] = ps_ctx.tile([65, 512], F32, tag="ctx",
                                                 name=f"ctxa_{g}{qc}")
                    ctxps[(qc, 1)] = ps_ctx.tile([65, 512], F32, tag="ctx",
                                                 name=f"ctxb_{g}{qc}")
                for kt in range(NKT):
                    ksl = slice(kt * 128, (kt + 1) * 128)
                    vbase = (g * NKT + kt) * VSTRIDE
                    scs = {}
                    for qc in qcs:
                        qsl = slice(qc * 512, (qc + 1) * 512)
                        scps = ps_sc.tile([128, 1024], F32, tag="sc",
                                          name=f"sc_{g}{qc}{kt}")
                        scs[qc] = scps
                    # QK: stationary K tile reused across both q-chunks
                    for h, psl in ((0, slice(0, 64)), (1, slice(64, 128))):
                        for qc in qcs:
                            qsl = slice(qc * 512, (qc + 1) * 512)
                            nc.tensor.matmul(
                                scs[qc][:, h * 512:(h + 1) * 512],
                                krot[psl, ksl], qrot[psl, qsl],
                                start=True, stop=True,
                                tile_position=(h * 64, 0))
                    for qc in qcs:
                        expt = exppool.tile([128, 1024], F32R, tag="expt",
                                            name=f"ex_{g}{qc}{kt}")
                        nc.scalar.activation(
                            expt[:], scs[qc][:],
                            mybir.ActivationFunctionType.Exp,
                            scale=float(HD) ** -0.5)
                        scs[qc] = expt
                    # PV: stationary V_aug tile reused across both q-chunks
                    for h in range(2):
                        vsl = slice(vbase + h * 65, vbase + (h + 1) * 65)
                        for qc in qcs:
                            nc.tensor.matmul(
                                ctxps[(qc, h)][:], vaug[:, vsl],
                                scs[qc][:, h * 512:(h + 1) * 512],
                                start=(kt == 0), stop=(kt == NKT - 1))
                for qc in qcs:
                    cua = ctxupool.tile([65, 512], BF16, tag="ctxu",
                                        name=f"cua_{g}{qc}")
                    cub = ctxupool.tile([65, 512], BF16, tag="ctxu",
                                        name=f"cub_{g}{qc}")
                    nc.vector.tensor_copy(cua[:], ctxps[(qc, 0)][:])
                    nc.vector.tensor_copy(cub[:], ctxps[(qc, 1)][:])
                    idx = g * 8 + qc * 2
                    nc.gpsimd.dma_start(out=den_dr[idx], in_=cua[64:65, :])
                    nc.gpsimd.dma_start(out=den_dr[idx + 1],
                                        in_=cub[64:65, :])
                    ctxus[(g, qc)] = (cua, cub)
                # per-pair normalize (denoms of both q-chunks at once)
                idx0 = g * 8 + qc0 * 2
                dp = sp3.tile([4, 512], BF16, tag="denpack",
                              name=f"dp{g}{qc0}")
                nc.sync.dma_start(out=dp[:],
                                  in_=den_dr.opt()[idx0:idx0 + 4, :])
                rc = sp3.tile([4, 512], F32, tag="recip",
                              name=f"rc{g}{qc0}")
                nc.vector.reciprocal(rc[:], dp[:])
                nc.sync.dma_start(out=rec_dr[idx0:idx0 + 4, :], in_=rc[:])
                for qc in qcs:
                    cua, cub = ctxus[(g, qc)]
                    qsl = slice(g * N + qc * 512, g * N + (qc + 1) * 512)
                    for h, cu, dst in ((0, cua, ctxn_a), (1, cub, ctxn_b)):
                        stg = sp3.tile([1, 512], F32, tag="stg3",
                                       name=f"st{g}{qc}{h}")
                        nc.sync.dma_start(
                            out=stg[:], in_=rec_dr[g * 8 + qc * 2 + h])
                        rep = sp3.tile([64, 512], F32, tag="rep",
                                       name=f"rp{g}{qc}{h}")
                        nc.gpsimd.partition_broadcast(rep[:], stg[:],
                                                      channels=64)
                        nc.vector.tensor_tensor(dst[:, qsl], cu[0:64, :],
                                                rep[:], MUL)

            # ---------------- pipelined emission ----------------
            with (
                tc.tile_pool(name="xtp", bufs=8) as xtpool,
                tc.tile_pool(name="vchp", bufs=3) as vchpool,
                tc.tile_pool(name="ps1", bufs=3, space="PSUM") as ps1,
                tc.tile_pool(name="ps1v", bufs=1, space="PSUM") as ps1v,
                tc.tile_pool(name="ps2", bufs=2, space="PSUM") as ps2,
            ):
                statps = {}
                for tname in ("q", "k"):
                    statps[(tname, 0)] = ps2.tile(
                        [40, 512], F32, tag="stat", name=f"stat_{tname}0")
                for r in range(4):
                    emit_proj_row(r, xtpool, vchpool, ps1, ps1v, statps)
                emit_statmath("q", 0, statps)
                emit_statmath("k", 0, statps)
                # batch-0 LN interleaved with batch-1 projections
                for tname in ("q", "k"):
                    statps[(tname, 1)] = ps2.tile(
                        [40, 512], F32, tag="stat", name=f"stat_{tname}1")
                for jj in range(4):
                    emit_proj_row(4 + jj, xtpool, vchpool, ps1, ps1v, statps)
                    emit_apply("q", 0, jj, wlnq_sb, blnq_sb)
                    emit_apply("k", 0, jj, wlnk_sb, blnk_sb)
                emit_statmath("q", 1, statps)
                emit_statmath("k", 1, statps)

            # batch-1 LN interleaved with batch-0 SDPA
            with (
                tc.tile_pool(name="expp", bufs=3) as exppool,
                tc.tile_pool(name="ctxup", bufs=4) as ctxupool,
                tc.tile_pool(name="sp3", bufs=2) as sp3,
                tc.tile_pool(name="ps_sc", bufs=2, space="PSUM") as ps_sc,
                tc.tile_pool(name="ps_ctx", bufs=4, space="PSUM") as ps_ctx,
            ):
                emit_apply("q", 1, 0, wlnq_sb, blnq_sb)
                emit_apply("k", 1, 0, wlnk_sb, blnk_sb)
                emit_apply("q", 1, 1, wlnq_sb, blnq_sb)
                emit_sdpa_qcpair(0, 0, exppool, ctxupool, sp3, ps_sc,
                                 ps_ctx)
                emit_apply("k", 1, 1, wlnk_sb, blnk_sb)
                emit_apply("q", 1, 2, wlnq_sb, blnq_sb)
                emit_apply("k", 1, 2, wlnk_sb, blnk_sb)
                emit_sdpa_qcpair(0, 2, exppool, ctxupool, sp3, ps_sc,
                                 ps_ctx)
                emit_apply("q", 1, 3, wlnq_sb, blnq_sb)
                emit_apply("k", 1, 3, wlnk_sb, blnk_sb)
                emit_sdpa_qcpair(1, 0, exppool, ctxupool, sp3, ps_sc,
                                 ps_ctx)
                emit_sdpa_qcpair(1, 2, exppool, ctxupool, sp3, ps_sc,
                                 ps_ctx)

            # ================= AllToAll (bf16) =================
            a2a_in = dpool.tile([NCORE, 128, 512], BF16)
            a2a_out = dpool.tile([NCORE, 128, 512], BF16)
            for j in range(NCORE):
                nc.gpsimd.dma_start(out=a2a_in[j][0:64, :],
                                    in_=ctxn_a[:, j * 512:(j + 1) * 512])
                nc.gpsimd.dma_start(out=a2a_in[j][64:128, :],
                                    in_=ctxn_b[:, j * 512:(j + 1) * 512])
            nc.gpsimd.collective_compute(
                "AllToAll", mybir.AluOpType.bypass,
                ins=[a2a_in.opt()], outs=[a2a_out.opt()],
                replica_groups=[list(range(NCORE))],
            )

            # ================= output projection (bf16) ==============
            with (
                tc.tile_pool(name="wop", bufs=3) as wopool,
                tc.tile_pool(name="sp5", bufs=4) as sp5,
                tc.tile_pool(name="ps_out", bufs=4, space="PSUM") as ps_out,
            ):
                ops = [ps_out.tile([128, 1024], F32, tag="outp",
                                   name=f"outp{i}") for i in range(4)]
                for kt in range(KT_DIM):
                    wo_sb = wopool.tile([128, DIM], BF16, tag="wo",
                                        name=f"wo{kt}")
                    nc.sync.dma_start(
                        out=wo_sb[:],
                        in_=wo_d.ap()[kt * 128:(kt + 1) * 128, :])
                    cg = wopool.tile([128, 512], BF16, tag="ctxg",
                                     name=f"cg{kt}")
                    nc.sync.dma_start(out=cg[:], in_=a2a_out[kt])
                    for rt in range(4):
                        for nh in range(2):
                            nc.tensor.matmul(
                                ops[rt][:, nh * 512:(nh + 1) * 512],
                                cg[:, rt * 128:(rt + 1) * 128],
                                wo_sb[:, nh * 512:(nh + 1) * 512],
                                start=(kt == 0), stop=(kt == KT_DIM - 1))
                for rt in range(4):
                    osb = sp5.tile([128, DIM], F32, tag="osb",
                                   name=f"osb{rt}")
                    nc.vector.tensor_tensor(osb[:], ops[rt][:], borep_sb[:],
                                            ADD)
                    nc.sync.dma_start(
                        out=out_d.ap()[rt * 128:(rt + 1) * 128, :],
                        in_=osb[:])

            if DEBUG_OUTPUTS:
                for g in range(B):
                    nc.sync.dma_start(
                        out=dbg_qrot.ap()[:, g * N:(g + 1) * N],
                        in_=qkt[("q", g)][:].bitcast(F32))
                    nc.sync.dma_start(
                        out=dbg_krot.ap()[:, g * N:(g + 1) * N],
                        in_=qkt[("k", g)][:].bitcast(F32))

            if DEBUG_OUTPUTS:
                nc.gpsimd.dma_start(out=dbg_den.ap()[:, :], in_=den_dr.opt())
                nc.gpsimd.dma_start(out=dbg_ctxn.ap()[0:64, :], in_=ctxn_a[:])
                nc.gpsimd.dma_start(out=dbg_ctxn.ap()[64:128, :],
                                    in_=ctxn_b[:])

    nc.compile()
    return nc


# ---------------------------------------------------------------- host side
def prepare_in_maps(x, rotary_cos, rotary_sin, Wq, bq, Wk, bk, Wv, bv,
                    q_norm_w, q_norm_b, k_norm_w, k_norm_b, Wo, bo):
    import ml_dtypes

    x = np.asarray(x, np.float32)
    xT = _round_fp32r(np.ascontiguousarray(x.reshape(R, DIM).T))

    Wcat = np.concatenate([np.asarray(Wq, np.float32),
                           np.asarray(Wk, np.float32),
                           np.asarray(Wv, np.float32)], axis=1)
    bcat = np.concatenate([np.asarray(bq, np.float32),
                           np.asarray(bk, np.float32),
                           np.asarray(bv, np.float32)])

    def head_cols(h, part):
        s = 192 * h + 64 * part
        return np.arange(s, s + 64)

    cos_flat = np.asarray(rotary_cos, np.float32).reshape(R, HD).T
    sin_flat = np.asarray(rotary_sin, np.float32).reshape(R, HD).T
    sinm = sin_flat.copy()
    sinm[0:32] = -sin_flat[0:32]
    cos_rep = np.ascontiguousarray(np.tile(cos_flat, (2, 1)))
    sinm_rep = np.ascontiguousarray(np.tile(sinm, (2, 1)))

    onesblk = np.zeros((RC, 2, 128, 40), np.float32)
    for j in range(RC):
        jj = j % 4
        onesblk[j, 0, 0:64, 2 * jj] = 1.0
        onesblk[j, 0, 64:128, 2 * jj + 1] = 1.0
        onesblk[j, 1, 0:64, 32 + 2 * jj] = 1.0
        onesblk[j, 1, 64:128, 32 + 2 * jj + 1] = 1.0

    wbln = np.stack([
        np.tile(np.asarray(q_norm_w, np.float32), 2)[:, None],
        np.tile(np.asarray(q_norm_b, np.float32), 2)[:, None],
        np.tile(np.asarray(k_norm_w, np.float32), 2)[:, None],
        np.tile(np.asarray(k_norm_b, np.float32), 2)[:, None],
    ])

    ident = np.eye(128, dtype=np.float32)
    ones64 = np.ones((128, 4 * NKT), np.float32)
    borep = np.tile(np.asarray(bo, np.float32)[None, :], (128, 1))
    wo_bf = np.asarray(Wo, np.float32).astype(ml_dtypes.bfloat16)

    in_maps = []
    for c in range(NCORE):
        hA, hB = 2 * c, 2 * c + 1
        cols = np.concatenate([
            head_cols(hA, 0), head_cols(hB, 0),
            head_cols(hA, 1), head_cols(hB, 1),
            head_cols(hA, 2), head_cols(hB, 2),
        ])
        wqkv_c = _round_fp32r(np.ascontiguousarray(Wcat[:, cols]))
        bqkv_c = np.ascontiguousarray(bcat[cols].reshape(3, 128, 1))
        in_maps.append({
            "xT": xT,
            "wqkv": wqkv_c,
            "bqkv": bqkv_c,
            "onesblk": onesblk,
            "wbln": wbln,
            "cosr": cos_rep,
            "sinm": sinm_rep,
            "ident": ident,
            "ones64": ones64,
            "wo": wo_bf,
            "borep": borep,
        })
    return in_maps


def assemble_output(results):
    out = np.empty((R, DIM), np.float32)
    for c in range(NCORE):
        out[c * 512:(c + 1) * 512] = results[c]["out"]
    return out.reshape(B, N, DIM)


_NC_CACHE = []


def kernel(**inputs) -> np.ndarray:
    if not _NC_CACHE:
        _NC_CACHE.append(build())
    nc = _NC_CACHE[0]
    in_maps = prepare_in_maps(**inputs)
    res = run_bass_kernel_spmd(nc, in_maps, core_ids=list(range(NCORE)))
    return assemble_output(res.results)
